# revision 24
# baseline (speedup 1.0000x reference)
"""Trainium2 Bass kernel for batched Jacobi iteration (5-point Laplacian).

Reference computation:
    x <- invD * (b - M x)   repeated `maxiter` times,
where M is the off-diagonal part of the 5-point Laplacian on a 512x512
grid, given in COO form.  For the actual inputs M is exactly the
4-neighbor stencil with value -1 and invD == 0.25, so the update is

    x_new[r, c] = 0.25 * (b[r, c] + x[r-1,c] + x[r+1,c] + x[r,c-1] + x[r,c+1])

(missing neighbors at grid edges contribute 0).

Strategy (8 NeuronCores, data parallel over batch B=16 -> 2 per core):
  - whole working set lives in SBUF for all iterations; state is fp16
  - grid stored as 4 "row planes" of (128 partitions=rows, 518 cols:
    2 zero pad cols each side so the interior starts 4B-aligned)
  - N/S coupling (+ optionally the b term) accumulates in PSUM via
    TensorE matmuls (tridiagonal / corner / identity stationaries,
    pre-scaled by 0.25)
  - E/W neighbor sum via shifted-AP adds on GpSimd/DVE
  - b term: `bk` planes injected via TensorE identity matmuls, the rest
    folded into a GpSimd scalar_tensor_tensor with precomputed 0.25*b
  - optional `sc` mode: ScalarE copies PSUM->SBUF fp16 so the DVE
    combine runs in 2x packed mode
  - iterations run in a HARDWARE loop (tc.For_i) with a small unroll, so
    the program size is O(1) in maxiter
"""

import sys

sys.path.insert(0, "/opt/trn_rl_repo")

import numpy as np

_N = 512  # grid side
_PL = 4  # row planes per grid
_P = 128  # partitions
_IC = 2  # interior start col (2 zero pad cols each side, 4B aligned)
_W = _N + 2 * _IC  # padded row width
_NCORES = 8
_BPC = 2  # batches per core

# ew: per-batch engine for the E/W shifted add: "dve" | "gp" | "tensor"
# unroll: iterations per hardware-loop trip
# bk: number of grid planes (of 8 total = 2 batches x 4) whose b term is
#     injected on TensorE; the rest fold into a GpSimd stt with bq=0.25*b
# sc: ScalarE copies PSUM->fp16 SBUF; DVE combine runs 2x in SBUF
CONFIG = {"ew": ("gp", "gp"), "unroll": 10, "hint": False, "sr": False,
          "split": True, "bk": 8, "sc": False, "fd": False, "ct": False}


def _build_nc(maxiter: int, ew_modes=("gp", "gp"), unroll=10, hint=False, sr=False,
              split=False, bk=8, sc=False, fd=False, ct=False):
    import concourse.bacc as bacc
    import concourse.mybir as mybir
    from concourse.tile import TileContext

    f32 = mybir.dt.float32
    f16 = mybir.dt.float16
    nc = bacc.Bacc("TRN2", target_bir_lowering=False, debug=False, num_devices=_NCORES)

    u_in = nc.declare_dram_parameter("u", [_BPC, _PL, _P, _N], f16, isOutput=False)
    b_in = nc.declare_dram_parameter("b", [_BPC, _PL, _P, _N], f16, isOutput=False)
    tm_in = nc.declare_dram_parameter("tm", [_P, _P], f32, isOutput=False)
    cn_in = nc.declare_dram_parameter("cn", [_P, _P], f32, isOutput=False)
    cs_in = nc.declare_dram_parameter("cs", [_P, _P], f32, isOutput=False)
    im_in = nc.declare_dram_parameter("im", [_P, _P], f32, isOutput=False)
    out = nc.declare_dram_parameter("out", [_BPC, _PL, _P, _N], f16, isOutput=True)

    trips = maxiter // unroll
    tail = maxiter % unroll

    # per-batch count of b-planes on TensorE (batch 0 filled first)
    bk0 = min(_PL, bk)
    bk1 = min(_PL, bk - bk0)
    bks = (bk0, bk1)

    with TileContext(nc) as tc:
        with (
            tc.tile_pool(name="const", bufs=1) as const,
            tc.tile_pool(name="state", bufs=1) as state,
            tc.tile_pool(name="work", bufs=1) as work,
            tc.tile_pool(name="psum", bufs=1, space="PSUM") as psum,
        ):
            # --- stationaries: load f32, convert to fp16 (entries 0/0.25, exact)
            tmf = const.tile([_P, _P], f32, tag="tmf")
            cnf = const.tile([_P, _P], f32, tag="cnf")
            csf = const.tile([_P, _P], f32, tag="csf")
            imf = const.tile([_P, _P], f32, tag="imf")
            nc.sync.dma_start(tmf[:], tm_in[:])
            nc.sync.dma_start(cnf[:], cn_in[:])
            nc.sync.dma_start(csf[:], cs_in[:])
            nc.sync.dma_start(imf[:], im_in[:])
            tm = const.tile([_P, _P], f16, tag="tm")
            cn = const.tile([_P, _P], f16, tag="cn")
            cs = const.tile([_P, _P], f16, tag="cs")
            im = const.tile([_P, _P], f16, tag="im")
            for dst, src in ((tm, tmf), (cn, cnf), (cs, csf), (im, imf)):
                nc.vector.tensor_copy(dst[:], src[:])

            # --- per-batch state
            xs, bs, ews, pcs, ps = [], [], [], [], []
            for bi in range(_BPC):
                xh = state.tile([_P, _PL, _W], f16, tag=f"x{bi}")
                nc.gpsimd.memset(xh[:], 0.0)
                for g in range(_PL):
                    nc.sync.dma_start(xh[:, g, _IC : _IC + _N], u_in[bi, g])
                xs.append(xh)

                bh = state.tile([_P, _PL, _N], f16, tag=f"b{bi}")
                for g in range(_PL):
                    nc.sync.dma_start(bh[:, g, :], b_in[bi, g])
                bs.append(bh)

                ew = state.tile([_P, _PL, _N], f16, tag=f"ew{bi}")
                ews.append(ew)
                if sc:
                    pc = state.tile([_P, _PL, _N], f16, tag=f"pc{bi}")
                    pcs.append(pc)
                if split:
                    pa = psum.tile([_P, 2, _N], f32, tag=f"p{bi}a")
                    pb = psum.tile([_P, 2, _N], f32, tag=f"p{bi}b")
                    p = [pa, pb]
                else:
                    p = psum.tile([_P, _PL, _N], f32, tag=f"p{bi}")
                ps.append(p)

            def _psum_dst(bi, g):
                if split:
                    return ps[bi][g // 2][:, g % 2, :]
                return ps[bi][:, g, :]

            def _plane_mms(bi, g):
                """list of (lhsT_ap, rhs_ap, out_partition_slice) per plane"""
                xh = xs[bi]
                mms = []
                if g < bks[bi]:
                    mms.append((im[:], bs[bi][:, g, :], None))
                if ew_modes[bi] == "tensor":
                    mms.append((im[:], xh[:, g, _IC - 1 : _IC - 1 + _N], None))
                    mms.append((im[:], xh[:, g, _IC + 1 : _IC + 1 + _N], None))
                mms.append((tm[:], xh[:, g, _IC : _IC + _N], None))
                if g > 0:
                    if ct:
                        # corner hits out partition 0 only: M=32 col-tile 0
                        mms.append((cn[:, 0:32], xh[:, g - 1, _IC : _IC + _N],
                                    (0, 32)))
                    else:
                        mms.append((cn[:], xh[:, g - 1, _IC : _IC + _N], None))
                if g < _PL - 1:
                    if ct:
                        # corner hits out partition 127: M=32 col-tile 3
                        mms.append((cs[:, 96:128], xh[:, g + 1, _IC : _IC + _N],
                                    (96, 128)))
                    else:
                        mms.append((cs[:], xh[:, g + 1, _IC : _IC + _N], None))
                return mms

            def mm_phase(bi, half=None):
                gs = range(_PL) if half is None else range(2 * half, 2 * half + 2)
                if not ct:
                    for g in gs:
                        mms = _plane_mms(bi, g)
                        for i, (mat, rhs, osl) in enumerate(mms):
                            dst = _psum_dst(bi, g)
                            if osl is not None:
                                dst = dst[osl[0] : osl[1]]
                            nc.tensor.matmul(
                                dst, mat, rhs,
                                start=(i == 0), stop=(i == len(mms) - 1),
                            )
                    return
                # ct: b-MMs first (full-width start), then the M=32 corner
                # MMs clustered as grp0/grp3 pairs so they run concurrently,
                # then tm-MMs (full-width stop closes each bank's group)
                seq = []  # (mat, rhs, osl, bank)
                mains = {g: [] for g in gs}
                corners = []
                for g in gs:
                    for mat, rhs, osl in _plane_mms(bi, g):
                        if osl is not None:
                            corners.append((mat, rhs, osl, g))
                        else:
                            mains[g].append((mat, rhs, None, g))
                # pair-order: alternate grp0 (cn, osl 0:32) / grp3 (cs 96:128)
                corners.sort(key=lambda e: (e[3] + (0 if e[2][0] == 0 else 1)))
                for g in gs:
                    seq += mains[g][:-1]  # b (and tensor-ew) MMs
                seq += corners
                for g in gs:
                    seq.append(mains[g][-1])  # tm closes the bank
                first = {}
                last = {}
                for i, e in enumerate(seq):
                    first.setdefault(e[3], i)
                    last[e[3]] = i
                for i, (mat, rhs, osl, g) in enumerate(seq):
                    dst = _psum_dst(bi, g)
                    kw = {}
                    if osl is not None:
                        dst = dst[osl[0] : osl[1]]
                        kw["tile_position"] = (0, osl[0])
                    nc.tensor.matmul(
                        dst, mat, rhs,
                        start=(i == first[g]), stop=(i == last[g]), **kw,
                    )

            def ew_phase(bi):
                xh = xs[bi]
                mode = ew_modes[bi]
                if mode == "tensor":
                    return
                eng = nc.vector if mode == "dve" else nc.gpsimd
                eng.tensor_add(
                    ews[bi][:], xh[:, :, _IC - 1 : _IC - 1 + _N],
                    xh[:, :, _IC + 1 : _IC + 1 + _N]
                )

            def bfold_phase(bi):
                """GpSimd: ew += b (in place) for planes whose b is folded;
                the fin stt's 0.25 scaling then covers b too."""
                k = bks[bi]
                if k >= _PL or ew_modes[bi] == "tensor":
                    return
                sl = slice(k, _PL)
                eng = nc.vector if fd else nc.gpsimd
                eng.tensor_add(
                    ews[bi][:, sl, :], ews[bi][:, sl, :], bs[bi][:, sl, :]
                )

            def sc_phase(bi, half=None):
                if not sc:
                    return
                if half is None:
                    nc.scalar.copy(pcs[bi][:], ps[bi][:])
                else:
                    sl = slice(2 * half, 2 * half + 2)
                    nc.scalar.copy(pcs[bi][:, sl, :], ps[bi][half][:])

            def fin_phase(bi, half=None):
                """combine into x: 0.25*ew + p (ew includes b on folded planes)"""
                xh = xs[bi]

                def _psrc(sl):
                    if sc:
                        return pcs[bi][:, sl, :]
                    if split:
                        h = sl.start // 2
                        return ps[bi][h][:, sl.start % 2 : sl.start % 2 + (sl.stop - sl.start), :]
                    return ps[bi][:, sl, :]

                lo, hi = (0, _PL) if half is None else (2 * half, 2 * half + 2)
                sl = slice(lo, hi)
                if ew_modes[bi] == "tensor":
                    nc.scalar.copy(xh[:, sl, _IC : _IC + _N], _psrc(sl))
                    return
                nc.vector.scalar_tensor_tensor(
                    xh[:, sl, _IC : _IC + _N], ews[bi][:, sl, :], 0.25,
                    _psrc(sl),
                    mybir.AluOpType.mult, mybir.AluOpType.add,
                )

            def body_once():
                for bi in range(_BPC):
                    ew_phase(bi)
                for bi in range(_BPC):
                    bfold_phase(bi)
                if split:
                    for bi in range(_BPC):
                        for h in range(2):
                            mm_phase(bi, h)
                    for bi in range(_BPC):
                        for h in range(2):
                            sc_phase(bi, h)
                            fin_phase(bi, h)
                else:
                    for bi in range(_BPC):
                        mm_phase(bi)
                    for bi in range(_BPC):
                        sc_phase(bi)
                        fin_phase(bi)

            if trips > 0:
                loop_kwargs = {}
                if hint:
                    loop_kwargs["hint_engines"] = (mybir.EngineType.PE,)
                if sr:
                    loop_kwargs["staggered_reset"] = True
                with tc.For_i(0, trips, 1, **loop_kwargs) as _i:
                    for _ in range(unroll):
                        body_once()
            for _ in range(tail):
                body_once()

            # --- writeback (fp16)
            for bi in range(_BPC):
                for g in range(_PL):
                    nc.sync.dma_start(out[bi, g], xs[bi][:, g, _IC : _IC + _N])

    nc.finalize()
    return nc


def parse_cfg(s: str) -> dict:
    """Parse 'ew0:ew1:unroll[:h][:s][:p][:sc][:bkN]' into a CONFIG dict."""
    parts = s.split(":")
    cfg = {"ew": (parts[0], parts[1]), "unroll": int(parts[2]), "hint": False,
           "sr": False, "split": False, "bk": 8, "sc": False, "fd": False,
           "ct": False}
    for p in parts[3:]:
        if p == "h":
            cfg["hint"] = True
        elif p == "s":
            cfg["sr"] = True
        elif p == "p":
            cfg["split"] = True
        elif p == "sc":
            cfg["sc"] = True
        elif p == "fd":
            cfg["fd"] = True
        elif p == "ct":
            cfg["ct"] = True
        elif p.startswith("bk"):
            cfg["bk"] = int(p[2:])
        else:
            raise ValueError(f"unknown config flag {p!r}")
    return cfg


_NC_CACHE: dict = {}


def _get_nc(maxiter: int):
    key = (maxiter, tuple(CONFIG["ew"]), CONFIG["unroll"],
           CONFIG.get("hint", False), CONFIG.get("sr", False),
           CONFIG.get("split", False), CONFIG.get("bk", 8),
           CONFIG.get("sc", False), CONFIG.get("fd", False),
           CONFIG.get("ct", False))
    if key not in _NC_CACHE:
        _NC_CACHE[key] = _build_nc(
            maxiter,
            ew_modes=tuple(CONFIG["ew"]),
            unroll=CONFIG["unroll"],
            hint=CONFIG.get("hint", False),
            sr=CONFIG.get("sr", False),
            split=CONFIG.get("split", False),
            bk=CONFIG.get("bk", 8),
            sc=CONFIG.get("sc", False),
            fd=CONFIG.get("fd", False),
            ct=CONFIG.get("ct", False),
        )
    return _NC_CACHE[key]


def _stencil_mats():
    # all stationaries pre-scaled by 0.25 (exact in fp16) so the PSUM
    # accumulator holds 0.25*(b + xN + xS) directly
    s = 0.25
    tm = np.zeros((_P, _P), np.float32)
    idx = np.arange(_P - 1)
    tm[idx, idx + 1] = s  # contribution of x[k] to out[k+1] (south nbr of k)
    tm[idx + 1, idx] = s  # north
    cn = np.zeros((_P, _P), np.float32)
    cn[_P - 1, 0] = s  # plane g-1 row 127 -> plane g row 0
    cs = np.zeros((_P, _P), np.float32)
    cs[0, _P - 1] = s  # plane g+1 row 0 -> plane g row 127
    im = s * np.eye(_P, dtype=np.float32)
    return tm, cn, cs, im


def _verify_stencil(M_rows, M_cols, M_vals, invD):
    """Check the COO matrix is exactly the uniform -1 4-neighbor stencil
    (no wraps) and invD == 0.25 everywhere."""
    r = np.asarray(M_rows).astype(np.int64)
    c = np.asarray(M_cols).astype(np.int64)
    v = np.asarray(M_vals)
    if not np.all(np.asarray(invD) == np.float32(0.25)):
        return False
    off = c - r
    bands = {}
    for o in (1, -1, _N, -_N):
        m = off == o
        bands[o] = m
    covered = bands[1] | bands[-1] | bands[_N] | bands[-_N]
    if not covered.all():
        return False
    # no row-wrap for the +-1 bands
    if np.any((r[bands[1]] % _N) == _N - 1) or np.any((r[bands[-1]] % _N) == 0):
        return False
    # each band must hit each eligible cell exactly once with value -1
    if not np.all(v == np.float32(-1.0)):
        return False
    n2 = _N * _N
    for o, m in bands.items():
        cnt = np.zeros(n2, np.int64)
        np.add.at(cnt, r[m], 1)
        rows2 = np.arange(n2)
        if o == 1:
            want = (rows2 % _N) != _N - 1
        elif o == -1:
            want = (rows2 % _N) != 0
        elif o == _N:
            want = rows2 < n2 - _N
        else:
            want = rows2 >= _N
        if not np.array_equal(cnt, want.astype(np.int64)):
            return False
    return True


def _fallback(u, b, M_rows, M_cols, M_vals, invD, maxiter):
    """Host scipy path — only taken if inputs are not the expected stencil."""
    from scipy.sparse import coo_matrix

    Bn = u.shape[0]
    n2 = _N * _N
    M = coo_matrix(
        (np.asarray(M_vals), (np.asarray(M_rows), np.asarray(M_cols))),
        shape=(n2, n2),
    ).tocsr()
    x = np.asarray(u).reshape(Bn, -1).astype(np.float32)
    bb = np.asarray(b).astype(np.float32)
    iD = np.asarray(invD).astype(np.float32)
    for _ in range(int(maxiter)):
        x = ((bb - (M @ x.T).T) * iD[None, :]).astype(np.float32)
    return x.reshape(u.shape)


TRACE = False
LAST = None  # BassKernelResults of the most recent run


def kernel(u, b, M_rows, M_cols, M_vals, invD, maxiter):
    global LAST
    from concourse.bass_utils import run_bass_kernel_spmd

    u = np.asarray(u)
    b = np.asarray(b)
    mi = int(maxiter)

    if not _verify_stencil(M_rows, M_cols, M_vals, invD):
        return _fallback(u, b, M_rows, M_cols, M_vals, invD, maxiter)

    nc = _get_nc(mi)
    tm, cn, cs, im = _stencil_mats()

    Bn = u.shape[0]
    assert Bn == _NCORES * _BPC
    u4 = np.ascontiguousarray(u.reshape(Bn, _PL, _P, _N).astype(np.float16))
    b4 = np.ascontiguousarray(b.reshape(Bn, _PL, _P, _N).astype(np.float16))

    in_maps = []
    for k in range(_NCORES):
        in_maps.append(
            {
                "u": u4[_BPC * k : _BPC * (k + 1)],
                "b": b4[_BPC * k : _BPC * (k + 1)],
                "tm": tm,
                "cn": cn,
                "cs": cs,
                "im": im,
            }
        )

    res = run_bass_kernel_spmd(nc, in_maps, list(range(_NCORES)), trace=TRACE)
    LAST = res
    outs = [res.results[k]["out"] for k in range(_NCORES)]
    full = np.concatenate(outs, axis=0).reshape(u.shape).astype(np.float32)
    return full


# revision 25
# speedup vs baseline: 1.1012x; 1.1012x over previous
"""Trainium2 Bass kernel for batched Jacobi iteration (5-point Laplacian).

Reference computation:
    x <- invD * (b - M x)   repeated `maxiter` times,
where M is the off-diagonal part of the 5-point Laplacian on a 512x512
grid, given in COO form.  For the actual inputs M is exactly the
4-neighbor stencil with value -1 and invD == 0.25, so the update is

    x_new[r, c] = 0.25 * (b[r, c] + x[r-1,c] + x[r+1,c] + x[r,c-1] + x[r,c+1])

(missing neighbors at grid edges contribute 0).

Strategy (8 NeuronCores, data parallel over batch B=16 -> 2 per core):
  - whole working set lives in SBUF for all iterations; state is fp16
  - grid stored as 4 "row planes" of (128 partitions=rows, 518 cols:
    2 zero pad cols each side so the interior starts 4B-aligned)
  - N/S coupling (+ optionally the b term) accumulates in PSUM via
    TensorE matmuls (tridiagonal / corner / identity stationaries,
    pre-scaled by 0.25)
  - E/W neighbor sum via shifted-AP adds on GpSimd/DVE
  - b term: `bk` planes injected via TensorE identity matmuls, the rest
    folded into a GpSimd scalar_tensor_tensor with precomputed 0.25*b
  - optional `sc` mode: ScalarE copies PSUM->SBUF fp16 so the DVE
    combine runs in 2x packed mode
  - iterations run in a HARDWARE loop (tc.For_i) with a small unroll, so
    the program size is O(1) in maxiter
"""

import sys

sys.path.insert(0, "/opt/trn_rl_repo")

import numpy as np

_N = 512  # grid side
_PL = 4  # row planes per grid
_P = 128  # partitions
_IC = 2  # interior start col (2 zero pad cols each side, 4B aligned)
_W = _N + 2 * _IC  # padded row width
_NCORES = 8
_BPC = 2  # batches per core

# ew: per-batch engine for the E/W shifted add: "dve" | "gp" | "tensor"
# unroll: iterations per hardware-loop trip
# bk: number of grid planes (of 8 total = 2 batches x 4) whose b term is
#     injected on TensorE; the rest fold into a GpSimd stt with bq=0.25*b
# sc: ScalarE copies PSUM->fp16 SBUF; DVE combine runs 2x in SBUF
CONFIG = {"ew": ("gp", "gp"), "unroll": 10, "hint": False, "sr": False,
          "split": True, "bk": 8, "sc": False, "fd": False, "ct": False}


def _build_nc(maxiter: int, ew_modes=("gp", "gp"), unroll=10, hint=False, sr=False,
              split=False, bk=8, sc=False, fd=False, ct=False):
    import concourse.bacc as bacc
    import concourse.mybir as mybir
    from concourse.tile import TileContext

    f32 = mybir.dt.float32
    f16 = mybir.dt.float16
    nc = bacc.Bacc("TRN2", target_bir_lowering=False, debug=False, num_devices=_NCORES)

    u_in = nc.declare_dram_parameter("u", [_BPC, _PL, _P, _N], f16, isOutput=False)
    b_in = nc.declare_dram_parameter("b", [_BPC, _PL, _P, _N], f16, isOutput=False)
    tm_in = nc.declare_dram_parameter("tm", [_P, _P], f32, isOutput=False)
    cn_in = nc.declare_dram_parameter("cn", [_P, _P], f32, isOutput=False)
    cs_in = nc.declare_dram_parameter("cs", [_P, _P], f32, isOutput=False)
    im_in = nc.declare_dram_parameter("im", [_P, _P], f32, isOutput=False)
    out = nc.declare_dram_parameter("out", [_BPC, _PL, _P, _N], f16, isOutput=True)

    trips = maxiter // unroll
    tail = maxiter % unroll

    # per-batch count of b-planes on TensorE (batch 0 filled first)
    bk0 = min(_PL, bk)
    bk1 = min(_PL, bk - bk0)
    bks = (bk0, bk1)

    with TileContext(nc) as tc:
        with (
            tc.tile_pool(name="const", bufs=1) as const,
            tc.tile_pool(name="state", bufs=1) as state,
            tc.tile_pool(name="work", bufs=1) as work,
            tc.tile_pool(name="ewp", bufs=2) as ewp,
            tc.tile_pool(name="psum", bufs=1, space="PSUM") as psum,
        ):
            # --- stationaries: load f32, convert to fp16 (entries 0/0.25, exact)
            tmf = const.tile([_P, _P], f32, tag="tmf")
            cnf = const.tile([_P, _P], f32, tag="cnf")
            csf = const.tile([_P, _P], f32, tag="csf")
            imf = const.tile([_P, _P], f32, tag="imf")
            nc.sync.dma_start(tmf[:], tm_in[:])
            nc.sync.dma_start(cnf[:], cn_in[:])
            nc.sync.dma_start(csf[:], cs_in[:])
            nc.sync.dma_start(imf[:], im_in[:])
            tm = const.tile([_P, _P], f16, tag="tm")
            cn = const.tile([_P, _P], f16, tag="cn")
            cs = const.tile([_P, _P], f16, tag="cs")
            im = const.tile([_P, _P], f16, tag="im")
            for dst, src in ((tm, tmf), (cn, cnf), (cs, csf), (im, imf)):
                nc.vector.tensor_copy(dst[:], src[:])

            # --- per-batch state
            xs, bs, ews, pcs, ps = [], [], [], [], []
            for bi in range(_BPC):
                xh = state.tile([_P, _PL, _W], f16, tag=f"x{bi}")
                nc.gpsimd.memset(xh[:], 0.0)
                for g in range(_PL):
                    nc.sync.dma_start(xh[:, g, _IC : _IC + _N], u_in[bi, g])
                xs.append(xh)

                bh = state.tile([_P, _PL, _N], f16, tag=f"b{bi}")
                for g in range(_PL):
                    nc.sync.dma_start(bh[:, g, :], b_in[bi, g])
                bs.append(bh)

                ews.append(None)  # allocated per-iteration from ewp
                if sc:
                    pc = state.tile([_P, _PL, _N], f16, tag=f"pc{bi}")
                    pcs.append(pc)
                if split:
                    pa = psum.tile([_P, 2, _N], f32, tag=f"p{bi}a")
                    pb = psum.tile([_P, 2, _N], f32, tag=f"p{bi}b")
                    p = [pa, pb]
                else:
                    p = psum.tile([_P, _PL, _N], f32, tag=f"p{bi}")
                ps.append(p)

            def _psum_dst(bi, g):
                if split:
                    return ps[bi][g // 2][:, g % 2, :]
                return ps[bi][:, g, :]

            def _plane_mms(bi, g):
                """list of (lhsT_ap, rhs_ap, out_partition_slice) per plane"""
                xh = xs[bi]
                mms = []
                if g < bks[bi]:
                    mms.append((im[:], bs[bi][:, g, :], None))
                if ew_modes[bi] == "tensor":
                    mms.append((im[:], xh[:, g, _IC - 1 : _IC - 1 + _N], None))
                    mms.append((im[:], xh[:, g, _IC + 1 : _IC + 1 + _N], None))
                mms.append((tm[:], xh[:, g, _IC : _IC + _N], None))
                if g > 0:
                    if ct:
                        # corner hits out partition 0 only: M=32 col-tile 0
                        mms.append((cn[:, 0:32], xh[:, g - 1, _IC : _IC + _N],
                                    (0, 32)))
                    else:
                        mms.append((cn[:], xh[:, g - 1, _IC : _IC + _N], None))
                if g < _PL - 1:
                    if ct:
                        # corner hits out partition 127: M=32 col-tile 3
                        mms.append((cs[:, 96:128], xh[:, g + 1, _IC : _IC + _N],
                                    (96, 128)))
                    else:
                        mms.append((cs[:], xh[:, g + 1, _IC : _IC + _N], None))
                return mms

            def mm_phase(bi, half=None):
                gs = range(_PL) if half is None else range(2 * half, 2 * half + 2)
                if not ct:
                    for g in gs:
                        mms = _plane_mms(bi, g)
                        for i, (mat, rhs, osl) in enumerate(mms):
                            dst = _psum_dst(bi, g)
                            if osl is not None:
                                dst = dst[osl[0] : osl[1]]
                            nc.tensor.matmul(
                                dst, mat, rhs,
                                start=(i == 0), stop=(i == len(mms) - 1),
                            )
                    return
                # ct: b-MMs first (full-width start), then the M=32 corner
                # MMs clustered as grp0/grp3 pairs so they run concurrently,
                # then tm-MMs (full-width stop closes each bank's group)
                seq = []  # (mat, rhs, osl, bank)
                mains = {g: [] for g in gs}
                corners = []
                for g in gs:
                    for mat, rhs, osl in _plane_mms(bi, g):
                        if osl is not None:
                            corners.append((mat, rhs, osl, g))
                        else:
                            mains[g].append((mat, rhs, None, g))
                # pair-order: alternate grp0 (cn, osl 0:32) / grp3 (cs 96:128)
                corners.sort(key=lambda e: (e[3] + (0 if e[2][0] == 0 else 1)))
                for g in gs:
                    seq += mains[g][:-1]  # b (and tensor-ew) MMs
                seq += corners
                for g in gs:
                    seq.append(mains[g][-1])  # tm closes the bank
                first = {}
                last = {}
                for i, e in enumerate(seq):
                    first.setdefault(e[3], i)
                    last[e[3]] = i
                for i, (mat, rhs, osl, g) in enumerate(seq):
                    dst = _psum_dst(bi, g)
                    kw = {}
                    if osl is not None:
                        dst = dst[osl[0] : osl[1]]
                        kw["tile_position"] = (0, osl[0])
                    nc.tensor.matmul(
                        dst, mat, rhs,
                        start=(i == first[g]), stop=(i == last[g]), **kw,
                    )

            def ew_phase(bi):
                xh = xs[bi]
                mode = ew_modes[bi]
                if mode == "tensor":
                    return
                ew = ewp.tile([_P, _PL, _N], f16, tag=f"ew{bi}")
                ews[bi] = ew
                eng = nc.vector if mode == "dve" else nc.gpsimd
                eng.tensor_add(
                    ew[:], xh[:, :, _IC - 1 : _IC - 1 + _N],
                    xh[:, :, _IC + 1 : _IC + 1 + _N]
                )

            def bfold_phase(bi):
                """GpSimd: ew += b (in place) for planes whose b is folded;
                the fin stt's 0.25 scaling then covers b too."""
                k = bks[bi]
                if k >= _PL or ew_modes[bi] == "tensor":
                    return
                sl = slice(k, _PL)
                eng = nc.vector if fd else nc.gpsimd
                eng.tensor_add(
                    ews[bi][:, sl, :], ews[bi][:, sl, :], bs[bi][:, sl, :]
                )

            def sc_phase(bi, half=None):
                if not sc:
                    return
                if half is None:
                    nc.scalar.copy(pcs[bi][:], ps[bi][:])
                else:
                    sl = slice(2 * half, 2 * half + 2)
                    nc.scalar.copy(pcs[bi][:, sl, :], ps[bi][half][:])

            def fin_phase(bi, half=None):
                """combine into x: 0.25*ew + p (ew includes b on folded planes)"""
                xh = xs[bi]

                def _psrc(sl):
                    if sc:
                        return pcs[bi][:, sl, :]
                    if split:
                        h = sl.start // 2
                        return ps[bi][h][:, sl.start % 2 : sl.start % 2 + (sl.stop - sl.start), :]
                    return ps[bi][:, sl, :]

                lo, hi = (0, _PL) if half is None else (2 * half, 2 * half + 2)
                sl = slice(lo, hi)
                if ew_modes[bi] == "tensor":
                    nc.scalar.copy(xh[:, sl, _IC : _IC + _N], _psrc(sl))
                    return
                nc.vector.scalar_tensor_tensor(
                    xh[:, sl, _IC : _IC + _N], ews[bi][:, sl, :], 0.25,
                    _psrc(sl),
                    mybir.AluOpType.mult, mybir.AluOpType.add,
                )

            def body_once():
                for bi in range(_BPC):
                    ew_phase(bi)
                for bi in range(_BPC):
                    bfold_phase(bi)
                if split:
                    for bi in range(_BPC):
                        for h in range(2):
                            mm_phase(bi, h)
                    for bi in range(_BPC):
                        for h in range(2):
                            sc_phase(bi, h)
                            fin_phase(bi, h)
                else:
                    for bi in range(_BPC):
                        mm_phase(bi)
                    for bi in range(_BPC):
                        sc_phase(bi)
                        fin_phase(bi)

            if trips > 0:
                loop_kwargs = {}
                if hint:
                    loop_kwargs["hint_engines"] = (mybir.EngineType.PE,)
                if sr:
                    loop_kwargs["staggered_reset"] = True
                with tc.For_i(0, trips, 1, **loop_kwargs) as _i:
                    for _ in range(unroll):
                        body_once()
            for _ in range(tail):
                body_once()

            # --- writeback (fp16)
            for bi in range(_BPC):
                for g in range(_PL):
                    nc.sync.dma_start(out[bi, g], xs[bi][:, g, _IC : _IC + _N])

    nc.finalize()
    return nc


def parse_cfg(s: str) -> dict:
    """Parse 'ew0:ew1:unroll[:h][:s][:p][:sc][:bkN]' into a CONFIG dict."""
    parts = s.split(":")
    cfg = {"ew": (parts[0], parts[1]), "unroll": int(parts[2]), "hint": False,
           "sr": False, "split": False, "bk": 8, "sc": False, "fd": False,
           "ct": False}
    for p in parts[3:]:
        if p == "h":
            cfg["hint"] = True
        elif p == "s":
            cfg["sr"] = True
        elif p == "p":
            cfg["split"] = True
        elif p == "sc":
            cfg["sc"] = True
        elif p == "fd":
            cfg["fd"] = True
        elif p == "ct":
            cfg["ct"] = True
        elif p.startswith("bk"):
            cfg["bk"] = int(p[2:])
        else:
            raise ValueError(f"unknown config flag {p!r}")
    return cfg


_NC_CACHE: dict = {}


def _get_nc(maxiter: int):
    key = (maxiter, tuple(CONFIG["ew"]), CONFIG["unroll"],
           CONFIG.get("hint", False), CONFIG.get("sr", False),
           CONFIG.get("split", False), CONFIG.get("bk", 8),
           CONFIG.get("sc", False), CONFIG.get("fd", False),
           CONFIG.get("ct", False))
    if key not in _NC_CACHE:
        _NC_CACHE[key] = _build_nc(
            maxiter,
            ew_modes=tuple(CONFIG["ew"]),
            unroll=CONFIG["unroll"],
            hint=CONFIG.get("hint", False),
            sr=CONFIG.get("sr", False),
            split=CONFIG.get("split", False),
            bk=CONFIG.get("bk", 8),
            sc=CONFIG.get("sc", False),
            fd=CONFIG.get("fd", False),
            ct=CONFIG.get("ct", False),
        )
    return _NC_CACHE[key]


def _stencil_mats():
    # all stationaries pre-scaled by 0.25 (exact in fp16) so the PSUM
    # accumulator holds 0.25*(b + xN + xS) directly
    s = 0.25
    tm = np.zeros((_P, _P), np.float32)
    idx = np.arange(_P - 1)
    tm[idx, idx + 1] = s  # contribution of x[k] to out[k+1] (south nbr of k)
    tm[idx + 1, idx] = s  # north
    cn = np.zeros((_P, _P), np.float32)
    cn[_P - 1, 0] = s  # plane g-1 row 127 -> plane g row 0
    cs = np.zeros((_P, _P), np.float32)
    cs[0, _P - 1] = s  # plane g+1 row 0 -> plane g row 127
    im = s * np.eye(_P, dtype=np.float32)
    return tm, cn, cs, im


def _verify_stencil(M_rows, M_cols, M_vals, invD):
    """Check the COO matrix is exactly the uniform -1 4-neighbor stencil
    (no wraps) and invD == 0.25 everywhere."""
    r = np.asarray(M_rows).astype(np.int64)
    c = np.asarray(M_cols).astype(np.int64)
    v = np.asarray(M_vals)
    if not np.all(np.asarray(invD) == np.float32(0.25)):
        return False
    off = c - r
    bands = {}
    for o in (1, -1, _N, -_N):
        m = off == o
        bands[o] = m
    covered = bands[1] | bands[-1] | bands[_N] | bands[-_N]
    if not covered.all():
        return False
    # no row-wrap for the +-1 bands
    if np.any((r[bands[1]] % _N) == _N - 1) or np.any((r[bands[-1]] % _N) == 0):
        return False
    # each band must hit each eligible cell exactly once with value -1
    if not np.all(v == np.float32(-1.0)):
        return False
    n2 = _N * _N
    for o, m in bands.items():
        cnt = np.zeros(n2, np.int64)
        np.add.at(cnt, r[m], 1)
        rows2 = np.arange(n2)
        if o == 1:
            want = (rows2 % _N) != _N - 1
        elif o == -1:
            want = (rows2 % _N) != 0
        elif o == _N:
            want = rows2 < n2 - _N
        else:
            want = rows2 >= _N
        if not np.array_equal(cnt, want.astype(np.int64)):
            return False
    return True


def _fallback(u, b, M_rows, M_cols, M_vals, invD, maxiter):
    """Host scipy path — only taken if inputs are not the expected stencil."""
    from scipy.sparse import coo_matrix

    Bn = u.shape[0]
    n2 = _N * _N
    M = coo_matrix(
        (np.asarray(M_vals), (np.asarray(M_rows), np.asarray(M_cols))),
        shape=(n2, n2),
    ).tocsr()
    x = np.asarray(u).reshape(Bn, -1).astype(np.float32)
    bb = np.asarray(b).astype(np.float32)
    iD = np.asarray(invD).astype(np.float32)
    for _ in range(int(maxiter)):
        x = ((bb - (M @ x.T).T) * iD[None, :]).astype(np.float32)
    return x.reshape(u.shape)


TRACE = False
LAST = None  # BassKernelResults of the most recent run


def kernel(u, b, M_rows, M_cols, M_vals, invD, maxiter):
    global LAST
    from concourse.bass_utils import run_bass_kernel_spmd

    u = np.asarray(u)
    b = np.asarray(b)
    mi = int(maxiter)

    if not _verify_stencil(M_rows, M_cols, M_vals, invD):
        return _fallback(u, b, M_rows, M_cols, M_vals, invD, maxiter)

    nc = _get_nc(mi)
    tm, cn, cs, im = _stencil_mats()

    Bn = u.shape[0]
    assert Bn == _NCORES * _BPC
    u4 = np.ascontiguousarray(u.reshape(Bn, _PL, _P, _N).astype(np.float16))
    b4 = np.ascontiguousarray(b.reshape(Bn, _PL, _P, _N).astype(np.float16))

    in_maps = []
    for k in range(_NCORES):
        in_maps.append(
            {
                "u": u4[_BPC * k : _BPC * (k + 1)],
                "b": b4[_BPC * k : _BPC * (k + 1)],
                "tm": tm,
                "cn": cn,
                "cs": cs,
                "im": im,
            }
        )

    res = run_bass_kernel_spmd(nc, in_maps, list(range(_NCORES)), trace=TRACE)
    LAST = res
    outs = [res.results[k]["out"] for k in range(_NCORES)]
    full = np.concatenate(outs, axis=0).reshape(u.shape).astype(np.float32)
    return full


# revision 26
# speedup vs baseline: 1.1378x; 1.0333x over previous
"""Trainium2 Bass kernel for batched Jacobi iteration (5-point Laplacian).

Reference computation:
    x <- invD * (b - M x)   repeated `maxiter` times,
where M is the off-diagonal part of the 5-point Laplacian on a 512x512
grid, given in COO form.  For the actual inputs M is exactly the
4-neighbor stencil with value -1 and invD == 0.25, so the update is

    x_new[r, c] = 0.25 * (b[r, c] + x[r-1,c] + x[r+1,c] + x[r,c-1] + x[r,c+1])

(missing neighbors at grid edges contribute 0).

Strategy (8 NeuronCores, data parallel over batch B=16 -> 2 per core):
  - whole working set lives in SBUF for all iterations; state is fp16
  - grid stored as 4 "row planes" of (128 partitions=rows, 518 cols:
    2 zero pad cols each side so the interior starts 4B-aligned)
  - N/S coupling (+ optionally the b term) accumulates in PSUM via
    TensorE matmuls (tridiagonal / corner / identity stationaries,
    pre-scaled by 0.25)
  - E/W neighbor sum via shifted-AP adds on GpSimd/DVE, double-buffered
    (bufs=2) so iteration k+1's ew overlaps iteration k's DVE combine
  - b term: `bk` planes injected via TensorE identity matmuls, the rest
    folded into a GpSimd scalar_tensor_tensor with precomputed 0.25*b
  - optional `sc` mode: ScalarE copies PSUM->SBUF fp16 so the DVE
    combine runs in 2x packed mode
  - iterations run in a HARDWARE loop (tc.For_i) with a small unroll, so
    the program size is O(1) in maxiter
"""

import sys

sys.path.insert(0, "/opt/trn_rl_repo")

import numpy as np

_N = 512  # grid side
_PL = 4  # row planes per grid
_P = 128  # partitions
_IC = 2  # interior start col (2 zero pad cols each side, 4B aligned)
_W = _N + 2 * _IC  # padded row width
_NCORES = 8
_BPC = 2  # batches per core

# ew: per-batch engine for the E/W shifted add: "dve" | "gp" | "tensor"
# unroll: iterations per hardware-loop trip
# bk: number of grid planes (of 8 total = 2 batches x 4) whose b term is
#     injected on TensorE; the rest fold into a GpSimd stt with bq=0.25*b
# sc: ScalarE copies PSUM->fp16 SBUF; DVE combine runs 2x in SBUF
CONFIG = {"ew": ("gp", "gp"), "unroll": 10, "hint": False, "sr": False,
          "split": True, "bk": 8, "sc": False, "fd": False, "ct": False}


def _build_nc(maxiter: int, ew_modes=("gp", "gp"), unroll=10, hint=False, sr=False,
              split=False, bk=8, sc=False, fd=False, ct=False):
    import concourse.bacc as bacc
    import concourse.mybir as mybir
    from concourse.tile import TileContext

    f32 = mybir.dt.float32
    f16 = mybir.dt.float16
    nc = bacc.Bacc("TRN2", target_bir_lowering=False, debug=False, num_devices=_NCORES)

    u_in = nc.declare_dram_parameter("u", [_BPC, _PL, _P, _N], f16, isOutput=False)
    b_in = nc.declare_dram_parameter("b", [_BPC, _PL, _P, _N], f16, isOutput=False)
    tm_in = nc.declare_dram_parameter("tm", [_P, _P], f32, isOutput=False)
    cn_in = nc.declare_dram_parameter("cn", [_P, _P], f32, isOutput=False)
    cs_in = nc.declare_dram_parameter("cs", [_P, _P], f32, isOutput=False)
    im_in = nc.declare_dram_parameter("im", [_P, _P], f32, isOutput=False)
    out = nc.declare_dram_parameter("out", [_BPC, _PL, _P, _N], f16, isOutput=True)

    trips = maxiter // unroll
    tail = maxiter % unroll

    # per-batch count of b-planes on TensorE (batch 0 filled first)
    bk0 = min(_PL, bk)
    bk1 = min(_PL, bk - bk0)
    bks = (bk0, bk1)

    with TileContext(nc) as tc:
        with (
            tc.tile_pool(name="const", bufs=1) as const,
            tc.tile_pool(name="state", bufs=1) as state,
            tc.tile_pool(name="work", bufs=1) as work,
            tc.tile_pool(name="ewp", bufs=2) as ewp,
            tc.tile_pool(name="psum", bufs=1, space="PSUM") as psum,
        ):
            # --- stationaries: load f32, convert to fp16 (entries 0/0.25, exact)
            tmf = const.tile([_P, _P], f32, tag="tmf")
            cnf = const.tile([_P, _P], f32, tag="cnf")
            csf = const.tile([_P, _P], f32, tag="csf")
            imf = const.tile([_P, _P], f32, tag="imf")
            nc.sync.dma_start(tmf[:], tm_in[:])
            nc.sync.dma_start(cnf[:], cn_in[:])
            nc.sync.dma_start(csf[:], cs_in[:])
            nc.sync.dma_start(imf[:], im_in[:])
            tm = const.tile([_P, _P], f16, tag="tm")
            cn = const.tile([_P, _P], f16, tag="cn")
            cs = const.tile([_P, _P], f16, tag="cs")
            im = const.tile([_P, _P], f16, tag="im")
            for dst, src in ((tm, tmf), (cn, cnf), (cs, csf), (im, imf)):
                nc.vector.tensor_copy(dst[:], src[:])

            # --- per-batch state
            xs, bs, ews, pcs, ps = [], [], [], [], []
            for bi in range(_BPC):
                xh = state.tile([_P, _PL, _W], f16, tag=f"x{bi}")
                nc.gpsimd.memset(xh[:], 0.0)
                for g in range(_PL):
                    nc.sync.dma_start(xh[:, g, _IC : _IC + _N], u_in[bi, g])
                xs.append(xh)

                bh = state.tile([_P, _PL, _N], f16, tag=f"b{bi}")
                for g in range(_PL):
                    nc.sync.dma_start(bh[:, g, :], b_in[bi, g])
                bs.append(bh)

                ews.append(None)  # allocated per-iteration from ewp
                if sc:
                    pc = state.tile([_P, _PL, _N], f16, tag=f"pc{bi}")
                    pcs.append(pc)
                if split:
                    pa = psum.tile([_P, 2, _N], f32, tag=f"p{bi}a")
                    pb = psum.tile([_P, 2, _N], f32, tag=f"p{bi}b")
                    p = [pa, pb]
                else:
                    p = psum.tile([_P, _PL, _N], f32, tag=f"p{bi}")
                ps.append(p)

            def _psum_dst(bi, g):
                if split:
                    return ps[bi][g // 2][:, g % 2, :]
                return ps[bi][:, g, :]

            def _plane_mms(bi, g):
                """list of (lhsT_ap, rhs_ap, out_partition_slice) per plane"""
                xh = xs[bi]
                mms = []
                if g < bks[bi]:
                    mms.append((im[:], bs[bi][:, g, :], None))
                if ew_modes[bi] == "tensor":
                    mms.append((im[:], xh[:, g, _IC - 1 : _IC - 1 + _N], None))
                    mms.append((im[:], xh[:, g, _IC + 1 : _IC + 1 + _N], None))
                mms.append((tm[:], xh[:, g, _IC : _IC + _N], None))
                if g > 0:
                    if ct:
                        # corner hits out partition 0 only: M=32 col-tile 0
                        mms.append((cn[:, 0:32], xh[:, g - 1, _IC : _IC + _N],
                                    (0, 32)))
                    else:
                        mms.append((cn[:], xh[:, g - 1, _IC : _IC + _N], None))
                if g < _PL - 1:
                    if ct:
                        # corner hits out partition 127: M=32 col-tile 3
                        mms.append((cs[:, 96:128], xh[:, g + 1, _IC : _IC + _N],
                                    (96, 128)))
                    else:
                        mms.append((cs[:], xh[:, g + 1, _IC : _IC + _N], None))
                return mms

            def mm_phase(bi, half=None):
                gs = range(_PL) if half is None else range(2 * half, 2 * half + 2)
                if not ct:
                    for g in gs:
                        mms = _plane_mms(bi, g)
                        for i, (mat, rhs, osl) in enumerate(mms):
                            dst = _psum_dst(bi, g)
                            if osl is not None:
                                dst = dst[osl[0] : osl[1]]
                            nc.tensor.matmul(
                                dst, mat, rhs,
                                start=(i == 0), stop=(i == len(mms) - 1),
                            )
                    return
                # ct: b-MMs first (full-width start), then the M=32 corner
                # MMs clustered as grp0/grp3 pairs so they run concurrently,
                # then tm-MMs (full-width stop closes each bank's group)
                seq = []  # (mat, rhs, osl, bank)
                mains = {g: [] for g in gs}
                corners = []
                for g in gs:
                    for mat, rhs, osl in _plane_mms(bi, g):
                        if osl is not None:
                            corners.append((mat, rhs, osl, g))
                        else:
                            mains[g].append((mat, rhs, None, g))
                # pair-order: alternate grp0 (cn, osl 0:32) / grp3 (cs 96:128)
                corners.sort(key=lambda e: (e[3] + (0 if e[2][0] == 0 else 1)))
                for g in gs:
                    seq += mains[g][:-1]  # b (and tensor-ew) MMs
                seq += corners
                for g in gs:
                    seq.append(mains[g][-1])  # tm closes the bank
                first = {}
                last = {}
                for i, e in enumerate(seq):
                    first.setdefault(e[3], i)
                    last[e[3]] = i
                for i, (mat, rhs, osl, g) in enumerate(seq):
                    dst = _psum_dst(bi, g)
                    kw = {}
                    if osl is not None:
                        dst = dst[osl[0] : osl[1]]
                        kw["tile_position"] = (0, osl[0])
                    nc.tensor.matmul(
                        dst, mat, rhs,
                        start=(i == first[g]), stop=(i == last[g]), **kw,
                    )

            def ew_phase(bi):
                xh = xs[bi]
                mode = ew_modes[bi]
                if mode == "tensor":
                    return
                ew = ewp.tile([_P, _PL, _N], f16, tag=f"ew{bi}")
                ews[bi] = ew
                eng = nc.vector if mode == "dve" else nc.gpsimd
                eng.tensor_add(
                    ew[:], xh[:, :, _IC - 1 : _IC - 1 + _N],
                    xh[:, :, _IC + 1 : _IC + 1 + _N]
                )

            def bfold_phase(bi):
                """GpSimd: ew += b (in place) for planes whose b is folded;
                the fin stt's 0.25 scaling then covers b too."""
                k = bks[bi]
                if k >= _PL or ew_modes[bi] == "tensor":
                    return
                sl = slice(k, _PL)
                eng = nc.vector if fd else nc.gpsimd
                eng.tensor_add(
                    ews[bi][:, sl, :], ews[bi][:, sl, :], bs[bi][:, sl, :]
                )

            def sc_phase(bi, half=None):
                if not sc:
                    return
                if half is None:
                    nc.scalar.copy(pcs[bi][:], ps[bi][:])
                else:
                    sl = slice(2 * half, 2 * half + 2)
                    nc.scalar.copy(pcs[bi][:, sl, :], ps[bi][half][:])

            def fin_phase(bi, half=None):
                """combine into x: 0.25*ew + p (ew includes b on folded planes)"""
                xh = xs[bi]

                def _psrc(sl):
                    if sc:
                        return pcs[bi][:, sl, :]
                    if split:
                        h = sl.start // 2
                        return ps[bi][h][:, sl.start % 2 : sl.start % 2 + (sl.stop - sl.start), :]
                    return ps[bi][:, sl, :]

                lo, hi = (0, _PL) if half is None else (2 * half, 2 * half + 2)
                sl = slice(lo, hi)
                if ew_modes[bi] == "tensor":
                    nc.scalar.copy(xh[:, sl, _IC : _IC + _N], _psrc(sl))
                    return
                nc.vector.scalar_tensor_tensor(
                    xh[:, sl, _IC : _IC + _N], ews[bi][:, sl, :], 0.25,
                    _psrc(sl),
                    mybir.AluOpType.mult, mybir.AluOpType.add,
                )

            def body_once():
                for bi in range(_BPC):
                    ew_phase(bi)
                for bi in range(_BPC):
                    bfold_phase(bi)
                if split:
                    for bi in range(_BPC):
                        for h in range(2):
                            mm_phase(bi, h)
                    for bi in range(_BPC):
                        for h in range(2):
                            sc_phase(bi, h)
                            fin_phase(bi, h)
                else:
                    for bi in range(_BPC):
                        mm_phase(bi)
                    for bi in range(_BPC):
                        sc_phase(bi)
                        fin_phase(bi)

            if trips > 0:
                loop_kwargs = {}
                if hint:
                    loop_kwargs["hint_engines"] = (mybir.EngineType.PE,)
                if sr:
                    loop_kwargs["staggered_reset"] = True
                with tc.For_i(0, trips, 1, **loop_kwargs) as _i:
                    for _ in range(unroll):
                        body_once()
            for _ in range(tail):
                body_once()

            # --- writeback (fp16)
            for bi in range(_BPC):
                for g in range(_PL):
                    nc.sync.dma_start(out[bi, g], xs[bi][:, g, _IC : _IC + _N])

    nc.finalize()
    return nc


def parse_cfg(s: str) -> dict:
    """Parse 'ew0:ew1:unroll[:h][:s][:p][:sc][:bkN]' into a CONFIG dict."""
    parts = s.split(":")
    cfg = {"ew": (parts[0], parts[1]), "unroll": int(parts[2]), "hint": False,
           "sr": False, "split": False, "bk": 8, "sc": False, "fd": False,
           "ct": False}
    for p in parts[3:]:
        if p == "h":
            cfg["hint"] = True
        elif p == "s":
            cfg["sr"] = True
        elif p == "p":
            cfg["split"] = True
        elif p == "sc":
            cfg["sc"] = True
        elif p == "fd":
            cfg["fd"] = True
        elif p == "ct":
            cfg["ct"] = True
        elif p.startswith("bk"):
            cfg["bk"] = int(p[2:])
        else:
            raise ValueError(f"unknown config flag {p!r}")
    return cfg


_NC_CACHE: dict = {}


def _get_nc(maxiter: int):
    key = (maxiter, tuple(CONFIG["ew"]), CONFIG["unroll"],
           CONFIG.get("hint", False), CONFIG.get("sr", False),
           CONFIG.get("split", False), CONFIG.get("bk", 8),
           CONFIG.get("sc", False), CONFIG.get("fd", False),
           CONFIG.get("ct", False))
    if key not in _NC_CACHE:
        _NC_CACHE[key] = _build_nc(
            maxiter,
            ew_modes=tuple(CONFIG["ew"]),
            unroll=CONFIG["unroll"],
            hint=CONFIG.get("hint", False),
            sr=CONFIG.get("sr", False),
            split=CONFIG.get("split", False),
            bk=CONFIG.get("bk", 8),
            sc=CONFIG.get("sc", False),
            fd=CONFIG.get("fd", False),
            ct=CONFIG.get("ct", False),
        )
    return _NC_CACHE[key]


def _stencil_mats():
    # all stationaries pre-scaled by 0.25 (exact in fp16) so the PSUM
    # accumulator holds 0.25*(b + xN + xS) directly
    s = 0.25
    tm = np.zeros((_P, _P), np.float32)
    idx = np.arange(_P - 1)
    tm[idx, idx + 1] = s  # contribution of x[k] to out[k+1] (south nbr of k)
    tm[idx + 1, idx] = s  # north
    cn = np.zeros((_P, _P), np.float32)
    cn[_P - 1, 0] = s  # plane g-1 row 127 -> plane g row 0
    cs = np.zeros((_P, _P), np.float32)
    cs[0, _P - 1] = s  # plane g+1 row 0 -> plane g row 127
    im = s * np.eye(_P, dtype=np.float32)
    return tm, cn, cs, im


def _verify_stencil(M_rows, M_cols, M_vals, invD):
    """Check the COO matrix is exactly the uniform -1 4-neighbor stencil
    (no wraps) and invD == 0.25 everywhere."""
    r = np.asarray(M_rows).astype(np.int64)
    c = np.asarray(M_cols).astype(np.int64)
    v = np.asarray(M_vals)
    if not np.all(np.asarray(invD) == np.float32(0.25)):
        return False
    off = c - r
    bands = {}
    for o in (1, -1, _N, -_N):
        m = off == o
        bands[o] = m
    covered = bands[1] | bands[-1] | bands[_N] | bands[-_N]
    if not covered.all():
        return False
    # no row-wrap for the +-1 bands
    if np.any((r[bands[1]] % _N) == _N - 1) or np.any((r[bands[-1]] % _N) == 0):
        return False
    # each band must hit each eligible cell exactly once with value -1
    if not np.all(v == np.float32(-1.0)):
        return False
    n2 = _N * _N
    for o, m in bands.items():
        cnt = np.zeros(n2, np.int64)
        np.add.at(cnt, r[m], 1)
        rows2 = np.arange(n2)
        if o == 1:
            want = (rows2 % _N) != _N - 1
        elif o == -1:
            want = (rows2 % _N) != 0
        elif o == _N:
            want = rows2 < n2 - _N
        else:
            want = rows2 >= _N
        if not np.array_equal(cnt, want.astype(np.int64)):
            return False
    return True


def _fallback(u, b, M_rows, M_cols, M_vals, invD, maxiter):
    """Host scipy path — only taken if inputs are not the expected stencil."""
    from scipy.sparse import coo_matrix

    Bn = u.shape[0]
    n2 = _N * _N
    M = coo_matrix(
        (np.asarray(M_vals), (np.asarray(M_rows), np.asarray(M_cols))),
        shape=(n2, n2),
    ).tocsr()
    x = np.asarray(u).reshape(Bn, -1).astype(np.float32)
    bb = np.asarray(b).astype(np.float32)
    iD = np.asarray(invD).astype(np.float32)
    for _ in range(int(maxiter)):
        x = ((bb - (M @ x.T).T) * iD[None, :]).astype(np.float32)
    return x.reshape(u.shape)


TRACE = False
LAST = None  # BassKernelResults of the most recent run


def kernel(u, b, M_rows, M_cols, M_vals, invD, maxiter):
    global LAST
    from concourse.bass_utils import run_bass_kernel_spmd

    u = np.asarray(u)
    b = np.asarray(b)
    mi = int(maxiter)

    if not _verify_stencil(M_rows, M_cols, M_vals, invD):
        return _fallback(u, b, M_rows, M_cols, M_vals, invD, maxiter)

    nc = _get_nc(mi)
    tm, cn, cs, im = _stencil_mats()

    Bn = u.shape[0]
    assert Bn == _NCORES * _BPC
    u4 = np.ascontiguousarray(u.reshape(Bn, _PL, _P, _N).astype(np.float16))
    b4 = np.ascontiguousarray(b.reshape(Bn, _PL, _P, _N).astype(np.float16))

    in_maps = []
    for k in range(_NCORES):
        in_maps.append(
            {
                "u": u4[_BPC * k : _BPC * (k + 1)],
                "b": b4[_BPC * k : _BPC * (k + 1)],
                "tm": tm,
                "cn": cn,
                "cs": cs,
                "im": im,
            }
        )

    res = run_bass_kernel_spmd(nc, in_maps, list(range(_NCORES)), trace=TRACE)
    LAST = res
    outs = [res.results[k]["out"] for k in range(_NCORES)]
    full = np.concatenate(outs, axis=0).reshape(u.shape).astype(np.float32)
    return full


# revision 29
# speedup vs baseline: 1.1687x; 1.0271x over previous
"""Trainium2 Bass kernel for batched Jacobi iteration (5-point Laplacian).

Reference computation:
    x <- invD * (b - M x)   repeated `maxiter` times,
where M is the off-diagonal part of the 5-point Laplacian on a 512x512
grid, given in COO form.  For the actual inputs M is exactly the
4-neighbor stencil with value -1 and invD == 0.25, so the update is

    x_new[r, c] = 0.25 * (b[r, c] + x[r-1,c] + x[r+1,c] + x[r,c-1] + x[r,c+1])

(missing neighbors at grid edges contribute 0).

Strategy (8 NeuronCores, data parallel over batch B=16 -> 2 per core):
  - whole working set lives in SBUF for all iterations; state is fp16
  - grid stored as 4 "row planes" of (128 partitions=rows, 518 cols:
    2 zero pad cols each side so the interior starts 4B-aligned)
  - N/S coupling (+ optionally the b term) accumulates in PSUM via
    TensorE matmuls (tridiagonal / corner / identity stationaries,
    pre-scaled by 0.25)
  - E/W neighbor sum via shifted-AP adds on DVE (fp16 2x mode),
    double-buffered (bufs=2) so iteration k+1's ew overlaps iteration
    k's combine
  - b term: `bk` planes injected via TensorE identity matmuls, the rest
    folded into a GpSimd scalar_tensor_tensor with precomputed 0.25*b
  - optional `sc` mode: ScalarE copies PSUM->SBUF fp16 so the DVE
    combine runs in 2x packed mode
  - iterations run in a HARDWARE loop (tc.For_i) with a small unroll, so
    the program size is O(1) in maxiter
"""

import sys

sys.path.insert(0, "/opt/trn_rl_repo")

import numpy as np

_N = 512  # grid side
_PL = 4  # row planes per grid
_P = 128  # partitions
_IC = 2  # interior start col (2 zero pad cols each side, 4B aligned)
_W = _N + 2 * _IC  # padded row width
_NCORES = 8
_BPC = 2  # batches per core

# ew: per-batch engine for the E/W shifted add: "dve" | "gp" | "tensor"
# unroll: iterations per hardware-loop trip
# bk: number of grid planes (of 8 total = 2 batches x 4) whose b term is
#     injected on TensorE; the rest fold into a GpSimd stt with bq=0.25*b
# sc: ScalarE copies PSUM->fp16 SBUF; DVE combine runs 2x in SBUF
CONFIG = {"ew": ("dve", "dve"), "unroll": 10, "hint": False, "sr": False,
          "split": True, "bk": 8, "sc": False, "fd": False, "ct": False}


def _build_nc(maxiter: int, ew_modes=("gp", "gp"), unroll=10, hint=False, sr=False,
              split=False, bk=8, sc=False, fd=False, ct=False):
    import concourse.bacc as bacc
    import concourse.mybir as mybir
    from concourse.tile import TileContext

    f32 = mybir.dt.float32
    f16 = mybir.dt.float16
    nc = bacc.Bacc("TRN2", target_bir_lowering=False, debug=False, num_devices=_NCORES)

    u_in = nc.declare_dram_parameter("u", [_BPC, _PL, _P, _N], f16, isOutput=False)
    b_in = nc.declare_dram_parameter("b", [_BPC, _PL, _P, _N], f16, isOutput=False)
    tm_in = nc.declare_dram_parameter("tm", [_P, _P], f32, isOutput=False)
    cn_in = nc.declare_dram_parameter("cn", [_P, _P], f32, isOutput=False)
    cs_in = nc.declare_dram_parameter("cs", [_P, _P], f32, isOutput=False)
    im_in = nc.declare_dram_parameter("im", [_P, _P], f32, isOutput=False)
    out = nc.declare_dram_parameter("out", [_BPC, _PL, _P, _N], f16, isOutput=True)

    trips = maxiter // unroll
    tail = maxiter % unroll

    # per-batch count of b-planes on TensorE (batch 0 filled first)
    bk0 = min(_PL, bk)
    bk1 = min(_PL, bk - bk0)
    bks = (bk0, bk1)

    with TileContext(nc) as tc:
        with (
            tc.tile_pool(name="const", bufs=1) as const,
            tc.tile_pool(name="state", bufs=1) as state,
            tc.tile_pool(name="work", bufs=1) as work,
            tc.tile_pool(name="ewp", bufs=2) as ewp,
            tc.tile_pool(name="psum", bufs=1, space="PSUM") as psum,
        ):
            # --- stationaries: load f32, convert to fp16 (entries 0/0.25, exact)
            tmf = const.tile([_P, _P], f32, tag="tmf")
            cnf = const.tile([_P, _P], f32, tag="cnf")
            csf = const.tile([_P, _P], f32, tag="csf")
            imf = const.tile([_P, _P], f32, tag="imf")
            nc.sync.dma_start(tmf[:], tm_in[:])
            nc.sync.dma_start(cnf[:], cn_in[:])
            nc.sync.dma_start(csf[:], cs_in[:])
            nc.sync.dma_start(imf[:], im_in[:])
            tm = const.tile([_P, _P], f16, tag="tm")
            cn = const.tile([_P, _P], f16, tag="cn")
            cs = const.tile([_P, _P], f16, tag="cs")
            im = const.tile([_P, _P], f16, tag="im")
            for dst, src in ((tm, tmf), (cn, cnf), (cs, csf), (im, imf)):
                nc.vector.tensor_copy(dst[:], src[:])

            # --- per-batch state
            xs, bs, ews, pcs, ps = [], [], [], [], []
            for bi in range(_BPC):
                xh = state.tile([_P, _PL, _W], f16, tag=f"x{bi}")
                nc.gpsimd.memset(xh[:], 0.0)
                for g in range(_PL):
                    nc.sync.dma_start(xh[:, g, _IC : _IC + _N], u_in[bi, g])
                xs.append(xh)

                bh = state.tile([_P, _PL, _N], f16, tag=f"b{bi}")
                for g in range(_PL):
                    nc.sync.dma_start(bh[:, g, :], b_in[bi, g])
                bs.append(bh)

                ews.append(None)  # allocated per-iteration from ewp
                if sc:
                    pc = state.tile([_P, _PL, _N], f16, tag=f"pc{bi}")
                    pcs.append(pc)
                if split:
                    pa = psum.tile([_P, 2, _N], f32, tag=f"p{bi}a")
                    pb = psum.tile([_P, 2, _N], f32, tag=f"p{bi}b")
                    p = [pa, pb]
                else:
                    p = psum.tile([_P, _PL, _N], f32, tag=f"p{bi}")
                ps.append(p)

            def _psum_dst(bi, g):
                if split:
                    return ps[bi][g // 2][:, g % 2, :]
                return ps[bi][:, g, :]

            def _plane_mms(bi, g):
                """list of (lhsT_ap, rhs_ap, out_partition_slice) per plane"""
                xh = xs[bi]
                mms = []
                if g < bks[bi]:
                    mms.append((im[:], bs[bi][:, g, :], None))
                if ew_modes[bi] == "tensor":
                    mms.append((im[:], xh[:, g, _IC - 1 : _IC - 1 + _N], None))
                    mms.append((im[:], xh[:, g, _IC + 1 : _IC + 1 + _N], None))
                mms.append((tm[:], xh[:, g, _IC : _IC + _N], None))
                if g > 0:
                    if ct:
                        # corner hits out partition 0 only: M=32 col-tile 0
                        mms.append((cn[:, 0:32], xh[:, g - 1, _IC : _IC + _N],
                                    (0, 32)))
                    else:
                        mms.append((cn[:], xh[:, g - 1, _IC : _IC + _N], None))
                if g < _PL - 1:
                    if ct:
                        # corner hits out partition 127: M=32 col-tile 3
                        mms.append((cs[:, 96:128], xh[:, g + 1, _IC : _IC + _N],
                                    (96, 128)))
                    else:
                        mms.append((cs[:], xh[:, g + 1, _IC : _IC + _N], None))
                return mms

            def mm_phase(bi, half=None):
                gs = range(_PL) if half is None else range(2 * half, 2 * half + 2)
                if not ct:
                    for g in gs:
                        mms = _plane_mms(bi, g)
                        for i, (mat, rhs, osl) in enumerate(mms):
                            dst = _psum_dst(bi, g)
                            if osl is not None:
                                dst = dst[osl[0] : osl[1]]
                            nc.tensor.matmul(
                                dst, mat, rhs,
                                start=(i == 0), stop=(i == len(mms) - 1),
                            )
                    return
                # ct: b-MMs first (full-width start), then the M=32 corner
                # MMs clustered as grp0/grp3 pairs so they run concurrently,
                # then tm-MMs (full-width stop closes each bank's group)
                seq = []  # (mat, rhs, osl, bank)
                mains = {g: [] for g in gs}
                corners = []
                for g in gs:
                    for mat, rhs, osl in _plane_mms(bi, g):
                        if osl is not None:
                            corners.append((mat, rhs, osl, g))
                        else:
                            mains[g].append((mat, rhs, None, g))
                # pair-order: alternate grp0 (cn, osl 0:32) / grp3 (cs 96:128)
                corners.sort(key=lambda e: (e[3] + (0 if e[2][0] == 0 else 1)))
                for g in gs:
                    seq += mains[g][:-1]  # b (and tensor-ew) MMs
                seq += corners
                for g in gs:
                    seq.append(mains[g][-1])  # tm closes the bank
                first = {}
                last = {}
                for i, e in enumerate(seq):
                    first.setdefault(e[3], i)
                    last[e[3]] = i
                for i, (mat, rhs, osl, g) in enumerate(seq):
                    dst = _psum_dst(bi, g)
                    kw = {}
                    if osl is not None:
                        dst = dst[osl[0] : osl[1]]
                        kw["tile_position"] = (0, osl[0])
                    nc.tensor.matmul(
                        dst, mat, rhs,
                        start=(i == first[g]), stop=(i == last[g]), **kw,
                    )

            def ew_phase(bi):
                xh = xs[bi]
                mode = ew_modes[bi]
                if mode == "tensor":
                    return
                ew = ewp.tile([_P, _PL, _N], f16, tag=f"ew{bi}")
                ews[bi] = ew
                eng = nc.vector if mode == "dve" else nc.gpsimd
                eng.tensor_add(
                    ew[:], xh[:, :, _IC - 1 : _IC - 1 + _N],
                    xh[:, :, _IC + 1 : _IC + 1 + _N]
                )

            def bfold_phase(bi):
                """GpSimd: ew += b (in place) for planes whose b is folded;
                the fin stt's 0.25 scaling then covers b too."""
                k = bks[bi]
                if k >= _PL or ew_modes[bi] == "tensor":
                    return
                sl = slice(k, _PL)
                eng = nc.vector if fd else nc.gpsimd
                eng.tensor_add(
                    ews[bi][:, sl, :], ews[bi][:, sl, :], bs[bi][:, sl, :]
                )

            def sc_phase(bi, half=None):
                if not sc:
                    return
                if half is None:
                    nc.scalar.copy(pcs[bi][:], ps[bi][:])
                else:
                    sl = slice(2 * half, 2 * half + 2)
                    nc.scalar.copy(pcs[bi][:, sl, :], ps[bi][half][:])

            def fin_phase(bi, half=None):
                """combine into x: 0.25*ew + p (ew includes b on folded planes)"""
                xh = xs[bi]

                def _psrc(sl):
                    if sc:
                        return pcs[bi][:, sl, :]
                    if split:
                        h = sl.start // 2
                        return ps[bi][h][:, sl.start % 2 : sl.start % 2 + (sl.stop - sl.start), :]
                    return ps[bi][:, sl, :]

                lo, hi = (0, _PL) if half is None else (2 * half, 2 * half + 2)
                sl = slice(lo, hi)
                if ew_modes[bi] == "tensor":
                    nc.scalar.copy(xh[:, sl, _IC : _IC + _N], _psrc(sl))
                    return
                nc.vector.scalar_tensor_tensor(
                    xh[:, sl, _IC : _IC + _N], ews[bi][:, sl, :], 0.25,
                    _psrc(sl),
                    mybir.AluOpType.mult, mybir.AluOpType.add,
                )

            def body_once():
                for bi in range(_BPC):
                    ew_phase(bi)
                for bi in range(_BPC):
                    bfold_phase(bi)
                if split:
                    for bi in range(_BPC):
                        for h in range(2):
                            mm_phase(bi, h)
                    for bi in range(_BPC):
                        for h in range(2):
                            sc_phase(bi, h)
                            fin_phase(bi, h)
                else:
                    for bi in range(_BPC):
                        mm_phase(bi)
                    for bi in range(_BPC):
                        sc_phase(bi)
                        fin_phase(bi)

            if trips > 0:
                loop_kwargs = {}
                if hint:
                    loop_kwargs["hint_engines"] = (mybir.EngineType.PE,)
                if sr:
                    loop_kwargs["staggered_reset"] = True
                with tc.For_i(0, trips, 1, **loop_kwargs) as _i:
                    for _ in range(unroll):
                        body_once()
            for _ in range(tail):
                body_once()

            # --- writeback (fp16)
            for bi in range(_BPC):
                for g in range(_PL):
                    nc.sync.dma_start(out[bi, g], xs[bi][:, g, _IC : _IC + _N])

    nc.finalize()
    return nc


def parse_cfg(s: str) -> dict:
    """Parse 'ew0:ew1:unroll[:h][:s][:p][:sc][:bkN]' into a CONFIG dict."""
    parts = s.split(":")
    cfg = {"ew": (parts[0], parts[1]), "unroll": int(parts[2]), "hint": False,
           "sr": False, "split": False, "bk": 8, "sc": False, "fd": False,
           "ct": False}
    for p in parts[3:]:
        if p == "h":
            cfg["hint"] = True
        elif p == "s":
            cfg["sr"] = True
        elif p == "p":
            cfg["split"] = True
        elif p == "sc":
            cfg["sc"] = True
        elif p == "fd":
            cfg["fd"] = True
        elif p == "ct":
            cfg["ct"] = True
        elif p.startswith("bk"):
            cfg["bk"] = int(p[2:])
        else:
            raise ValueError(f"unknown config flag {p!r}")
    return cfg


_NC_CACHE: dict = {}


def _get_nc(maxiter: int):
    key = (maxiter, tuple(CONFIG["ew"]), CONFIG["unroll"],
           CONFIG.get("hint", False), CONFIG.get("sr", False),
           CONFIG.get("split", False), CONFIG.get("bk", 8),
           CONFIG.get("sc", False), CONFIG.get("fd", False),
           CONFIG.get("ct", False))
    if key not in _NC_CACHE:
        _NC_CACHE[key] = _build_nc(
            maxiter,
            ew_modes=tuple(CONFIG["ew"]),
            unroll=CONFIG["unroll"],
            hint=CONFIG.get("hint", False),
            sr=CONFIG.get("sr", False),
            split=CONFIG.get("split", False),
            bk=CONFIG.get("bk", 8),
            sc=CONFIG.get("sc", False),
            fd=CONFIG.get("fd", False),
            ct=CONFIG.get("ct", False),
        )
    return _NC_CACHE[key]


def _stencil_mats():
    # all stationaries pre-scaled by 0.25 (exact in fp16) so the PSUM
    # accumulator holds 0.25*(b + xN + xS) directly
    s = 0.25
    tm = np.zeros((_P, _P), np.float32)
    idx = np.arange(_P - 1)
    tm[idx, idx + 1] = s  # contribution of x[k] to out[k+1] (south nbr of k)
    tm[idx + 1, idx] = s  # north
    cn = np.zeros((_P, _P), np.float32)
    cn[_P - 1, 0] = s  # plane g-1 row 127 -> plane g row 0
    cs = np.zeros((_P, _P), np.float32)
    cs[0, _P - 1] = s  # plane g+1 row 0 -> plane g row 127
    im = s * np.eye(_P, dtype=np.float32)
    return tm, cn, cs, im


def _verify_stencil(M_rows, M_cols, M_vals, invD):
    """Check the COO matrix is exactly the uniform -1 4-neighbor stencil
    (no wraps) and invD == 0.25 everywhere."""
    r = np.asarray(M_rows).astype(np.int64)
    c = np.asarray(M_cols).astype(np.int64)
    v = np.asarray(M_vals)
    if not np.all(np.asarray(invD) == np.float32(0.25)):
        return False
    off = c - r
    bands = {}
    for o in (1, -1, _N, -_N):
        m = off == o
        bands[o] = m
    covered = bands[1] | bands[-1] | bands[_N] | bands[-_N]
    if not covered.all():
        return False
    # no row-wrap for the +-1 bands
    if np.any((r[bands[1]] % _N) == _N - 1) or np.any((r[bands[-1]] % _N) == 0):
        return False
    # each band must hit each eligible cell exactly once with value -1
    if not np.all(v == np.float32(-1.0)):
        return False
    n2 = _N * _N
    for o, m in bands.items():
        cnt = np.zeros(n2, np.int64)
        np.add.at(cnt, r[m], 1)
        rows2 = np.arange(n2)
        if o == 1:
            want = (rows2 % _N) != _N - 1
        elif o == -1:
            want = (rows2 % _N) != 0
        elif o == _N:
            want = rows2 < n2 - _N
        else:
            want = rows2 >= _N
        if not np.array_equal(cnt, want.astype(np.int64)):
            return False
    return True


def _fallback(u, b, M_rows, M_cols, M_vals, invD, maxiter):
    """Host scipy path — only taken if inputs are not the expected stencil."""
    from scipy.sparse import coo_matrix

    Bn = u.shape[0]
    n2 = _N * _N
    M = coo_matrix(
        (np.asarray(M_vals), (np.asarray(M_rows), np.asarray(M_cols))),
        shape=(n2, n2),
    ).tocsr()
    x = np.asarray(u).reshape(Bn, -1).astype(np.float32)
    bb = np.asarray(b).astype(np.float32)
    iD = np.asarray(invD).astype(np.float32)
    for _ in range(int(maxiter)):
        x = ((bb - (M @ x.T).T) * iD[None, :]).astype(np.float32)
    return x.reshape(u.shape)


TRACE = False
LAST = None  # BassKernelResults of the most recent run


def kernel(u, b, M_rows, M_cols, M_vals, invD, maxiter):
    global LAST
    from concourse.bass_utils import run_bass_kernel_spmd

    u = np.asarray(u)
    b = np.asarray(b)
    mi = int(maxiter)

    if not _verify_stencil(M_rows, M_cols, M_vals, invD):
        return _fallback(u, b, M_rows, M_cols, M_vals, invD, maxiter)

    nc = _get_nc(mi)
    tm, cn, cs, im = _stencil_mats()

    Bn = u.shape[0]
    assert Bn == _NCORES * _BPC
    u4 = np.ascontiguousarray(u.reshape(Bn, _PL, _P, _N).astype(np.float16))
    b4 = np.ascontiguousarray(b.reshape(Bn, _PL, _P, _N).astype(np.float16))

    in_maps = []
    for k in range(_NCORES):
        in_maps.append(
            {
                "u": u4[_BPC * k : _BPC * (k + 1)],
                "b": b4[_BPC * k : _BPC * (k + 1)],
                "tm": tm,
                "cn": cn,
                "cs": cs,
                "im": im,
            }
        )

    res = run_bass_kernel_spmd(nc, in_maps, list(range(_NCORES)), trace=TRACE)
    LAST = res
    outs = [res.results[k]["out"] for k in range(_NCORES)]
    full = np.concatenate(outs, axis=0).reshape(u.shape).astype(np.float32)
    return full


# revision 31
# speedup vs baseline: 1.2068x; 1.0326x over previous
"""Trainium2 Bass kernel for batched Jacobi iteration (5-point Laplacian).

Reference computation:
    x <- invD * (b - M x)   repeated `maxiter` times,
where M is the off-diagonal part of the 5-point Laplacian on a 512x512
grid, given in COO form.  For the actual inputs M is exactly the
4-neighbor stencil with value -1 and invD == 0.25, so the update is

    x_new[r, c] = 0.25 * (b[r, c] + x[r-1,c] + x[r+1,c] + x[r,c-1] + x[r,c+1])

(missing neighbors at grid edges contribute 0).

Strategy (8 NeuronCores, data parallel over batch B=16 -> 2 per core):
  - whole working set lives in SBUF for all iterations; state is fp16
  - grid stored as 4 "row planes" of (128 partitions=rows, 518 cols:
    2 zero pad cols each side so the interior starts 4B-aligned)
  - N/S coupling (+ optionally the b term) accumulates in PSUM via
    TensorE matmuls (tridiagonal / corner / identity stationaries,
    pre-scaled by 0.25)
  - E/W neighbor sum via shifted-AP adds split across DVE (planes 0-1,
    fp16 2x mode) and GpSimd (planes 2-3), double-buffered (bufs=2) so
    iteration k+1's ew overlaps iteration k's combine
  - b term: `bk` planes injected via TensorE identity matmuls, the rest
    folded into a GpSimd scalar_tensor_tensor with precomputed 0.25*b
  - optional `sc` mode: ScalarE copies PSUM->SBUF fp16 so the DVE
    combine runs in 2x packed mode
  - iterations run in a HARDWARE loop (tc.For_i) with a small unroll, so
    the program size is O(1) in maxiter
"""

import sys

sys.path.insert(0, "/opt/trn_rl_repo")

import numpy as np

_N = 512  # grid side
_PL = 4  # row planes per grid
_P = 128  # partitions
_IC = 2  # interior start col (2 zero pad cols each side, 4B aligned)
_W = _N + 2 * _IC  # padded row width
_NCORES = 8
_BPC = 2  # batches per core

# ew: per-batch engine for the E/W shifted add: "dve" | "gp" | "tensor"
# unroll: iterations per hardware-loop trip
# bk: number of grid planes (of 8 total = 2 batches x 4) whose b term is
#     injected on TensorE; the rest fold into a GpSimd stt with bq=0.25*b
# sc: ScalarE copies PSUM->fp16 SBUF; DVE combine runs 2x in SBUF
CONFIG = {"ew": ("mix", "mix"), "unroll": 10, "hint": False, "sr": False,
          "split": True, "bk": 8, "sc": False, "fd": False, "ct": False}


def _build_nc(maxiter: int, ew_modes=("gp", "gp"), unroll=10, hint=False, sr=False,
              split=False, bk=8, sc=False, fd=False, ct=False):
    import concourse.bacc as bacc
    import concourse.mybir as mybir
    from concourse.tile import TileContext

    f32 = mybir.dt.float32
    f16 = mybir.dt.float16
    nc = bacc.Bacc("TRN2", target_bir_lowering=False, debug=False, num_devices=_NCORES)

    u_in = nc.declare_dram_parameter("u", [_BPC, _PL, _P, _N], f16, isOutput=False)
    b_in = nc.declare_dram_parameter("b", [_BPC, _PL, _P, _N], f16, isOutput=False)
    tm_in = nc.declare_dram_parameter("tm", [_P, _P], f32, isOutput=False)
    cn_in = nc.declare_dram_parameter("cn", [_P, _P], f32, isOutput=False)
    cs_in = nc.declare_dram_parameter("cs", [_P, _P], f32, isOutput=False)
    im_in = nc.declare_dram_parameter("im", [_P, _P], f32, isOutput=False)
    out = nc.declare_dram_parameter("out", [_BPC, _PL, _P, _N], f16, isOutput=True)

    trips = maxiter // unroll
    tail = maxiter % unroll

    # per-batch count of b-planes on TensorE (batch 0 filled first)
    bk0 = min(_PL, bk)
    bk1 = min(_PL, bk - bk0)
    bks = (bk0, bk1)

    with TileContext(nc) as tc:
        with (
            tc.tile_pool(name="const", bufs=1) as const,
            tc.tile_pool(name="state", bufs=1) as state,
            tc.tile_pool(name="work", bufs=1) as work,
            tc.tile_pool(name="ewp", bufs=2) as ewp,
            tc.tile_pool(name="psum", bufs=1, space="PSUM") as psum,
        ):
            # --- stationaries: load f32, convert to fp16 (entries 0/0.25, exact)
            tmf = const.tile([_P, _P], f32, tag="tmf")
            cnf = const.tile([_P, _P], f32, tag="cnf")
            csf = const.tile([_P, _P], f32, tag="csf")
            imf = const.tile([_P, _P], f32, tag="imf")
            nc.sync.dma_start(tmf[:], tm_in[:])
            nc.sync.dma_start(cnf[:], cn_in[:])
            nc.sync.dma_start(csf[:], cs_in[:])
            nc.sync.dma_start(imf[:], im_in[:])
            tm = const.tile([_P, _P], f16, tag="tm")
            cn = const.tile([_P, _P], f16, tag="cn")
            cs = const.tile([_P, _P], f16, tag="cs")
            im = const.tile([_P, _P], f16, tag="im")
            for dst, src in ((tm, tmf), (cn, cnf), (cs, csf), (im, imf)):
                nc.vector.tensor_copy(dst[:], src[:])

            # --- per-batch state
            xs, bs, ews, pcs, ps = [], [], [], [], []
            for bi in range(_BPC):
                xh = state.tile([_P, _PL, _W], f16, tag=f"x{bi}")
                nc.gpsimd.memset(xh[:], 0.0)
                for g in range(_PL):
                    nc.sync.dma_start(xh[:, g, _IC : _IC + _N], u_in[bi, g])
                xs.append(xh)

                bh = state.tile([_P, _PL, _N], f16, tag=f"b{bi}")
                for g in range(_PL):
                    nc.sync.dma_start(bh[:, g, :], b_in[bi, g])
                bs.append(bh)

                ews.append(None)  # allocated per-iteration from ewp
                if sc:
                    pc = state.tile([_P, _PL, _N], f16, tag=f"pc{bi}")
                    pcs.append(pc)
                if split:
                    pa = psum.tile([_P, 2, _N], f32, tag=f"p{bi}a")
                    pb = psum.tile([_P, 2, _N], f32, tag=f"p{bi}b")
                    p = [pa, pb]
                else:
                    p = psum.tile([_P, _PL, _N], f32, tag=f"p{bi}")
                ps.append(p)

            def _psum_dst(bi, g):
                if split:
                    return ps[bi][g // 2][:, g % 2, :]
                return ps[bi][:, g, :]

            def _plane_mms(bi, g):
                """list of (lhsT_ap, rhs_ap, out_partition_slice) per plane"""
                xh = xs[bi]
                mms = []
                if g < bks[bi]:
                    mms.append((im[:], bs[bi][:, g, :], None))
                if ew_modes[bi] == "tensor":
                    mms.append((im[:], xh[:, g, _IC - 1 : _IC - 1 + _N], None))
                    mms.append((im[:], xh[:, g, _IC + 1 : _IC + 1 + _N], None))
                mms.append((tm[:], xh[:, g, _IC : _IC + _N], None))
                if g > 0:
                    if ct:
                        # corner hits out partition 0 only: M=32 col-tile 0
                        mms.append((cn[:, 0:32], xh[:, g - 1, _IC : _IC + _N],
                                    (0, 32)))
                    else:
                        mms.append((cn[:], xh[:, g - 1, _IC : _IC + _N], None))
                if g < _PL - 1:
                    if ct:
                        # corner hits out partition 127: M=32 col-tile 3
                        mms.append((cs[:, 96:128], xh[:, g + 1, _IC : _IC + _N],
                                    (96, 128)))
                    else:
                        mms.append((cs[:], xh[:, g + 1, _IC : _IC + _N], None))
                return mms

            def mm_phase(bi, half=None):
                gs = range(_PL) if half is None else range(2 * half, 2 * half + 2)
                if not ct:
                    for g in gs:
                        mms = _plane_mms(bi, g)
                        for i, (mat, rhs, osl) in enumerate(mms):
                            dst = _psum_dst(bi, g)
                            if osl is not None:
                                dst = dst[osl[0] : osl[1]]
                            nc.tensor.matmul(
                                dst, mat, rhs,
                                start=(i == 0), stop=(i == len(mms) - 1),
                            )
                    return
                # ct: b-MMs first (full-width start), then the M=32 corner
                # MMs clustered as grp0/grp3 pairs so they run concurrently,
                # then tm-MMs (full-width stop closes each bank's group)
                seq = []  # (mat, rhs, osl, bank)
                mains = {g: [] for g in gs}
                corners = []
                for g in gs:
                    for mat, rhs, osl in _plane_mms(bi, g):
                        if osl is not None:
                            corners.append((mat, rhs, osl, g))
                        else:
                            mains[g].append((mat, rhs, None, g))
                # pair-order: alternate grp0 (cn, osl 0:32) / grp3 (cs 96:128)
                corners.sort(key=lambda e: (e[3] + (0 if e[2][0] == 0 else 1)))
                for g in gs:
                    seq += mains[g][:-1]  # b (and tensor-ew) MMs
                seq += corners
                for g in gs:
                    seq.append(mains[g][-1])  # tm closes the bank
                first = {}
                last = {}
                for i, e in enumerate(seq):
                    first.setdefault(e[3], i)
                    last[e[3]] = i
                for i, (mat, rhs, osl, g) in enumerate(seq):
                    dst = _psum_dst(bi, g)
                    kw = {}
                    if osl is not None:
                        dst = dst[osl[0] : osl[1]]
                        kw["tile_position"] = (0, osl[0])
                    nc.tensor.matmul(
                        dst, mat, rhs,
                        start=(i == first[g]), stop=(i == last[g]), **kw,
                    )

            def ew_phase(bi):
                xh = xs[bi]
                mode = ew_modes[bi]
                if mode == "tensor":
                    return
                ew = ewp.tile([_P, _PL, _N], f16, tag=f"ew{bi}")
                ews[bi] = ew
                if mode == "mix":
                    # planes 0-1 on DVE (fast 2x path), 2-3 on idle GpSimd
                    nc.vector.tensor_add(
                        ew[:, 0:2, :], xh[:, 0:2, _IC - 1 : _IC - 1 + _N],
                        xh[:, 0:2, _IC + 1 : _IC + 1 + _N]
                    )
                    nc.gpsimd.tensor_add(
                        ew[:, 2:4, :], xh[:, 2:4, _IC - 1 : _IC - 1 + _N],
                        xh[:, 2:4, _IC + 1 : _IC + 1 + _N]
                    )
                    return
                eng = nc.vector if mode == "dve" else nc.gpsimd
                eng.tensor_add(
                    ew[:], xh[:, :, _IC - 1 : _IC - 1 + _N],
                    xh[:, :, _IC + 1 : _IC + 1 + _N]
                )

            def bfold_phase(bi):
                """GpSimd: ew += b (in place) for planes whose b is folded;
                the fin stt's 0.25 scaling then covers b too."""
                k = bks[bi]
                if k >= _PL or ew_modes[bi] == "tensor":
                    return
                sl = slice(k, _PL)
                eng = nc.vector if fd else nc.gpsimd
                eng.tensor_add(
                    ews[bi][:, sl, :], ews[bi][:, sl, :], bs[bi][:, sl, :]
                )

            def sc_phase(bi, half=None):
                if not sc:
                    return
                if half is None:
                    nc.scalar.copy(pcs[bi][:], ps[bi][:])
                else:
                    sl = slice(2 * half, 2 * half + 2)
                    nc.scalar.copy(pcs[bi][:, sl, :], ps[bi][half][:])

            def fin_phase(bi, half=None):
                """combine into x: 0.25*ew + p (ew includes b on folded planes)"""
                xh = xs[bi]

                def _psrc(sl):
                    if sc:
                        return pcs[bi][:, sl, :]
                    if split:
                        h = sl.start // 2
                        return ps[bi][h][:, sl.start % 2 : sl.start % 2 + (sl.stop - sl.start), :]
                    return ps[bi][:, sl, :]

                lo, hi = (0, _PL) if half is None else (2 * half, 2 * half + 2)
                sl = slice(lo, hi)
                if ew_modes[bi] == "tensor":
                    nc.scalar.copy(xh[:, sl, _IC : _IC + _N], _psrc(sl))
                    return
                nc.vector.scalar_tensor_tensor(
                    xh[:, sl, _IC : _IC + _N], ews[bi][:, sl, :], 0.25,
                    _psrc(sl),
                    mybir.AluOpType.mult, mybir.AluOpType.add,
                )

            def body_once():
                for bi in range(_BPC):
                    ew_phase(bi)
                for bi in range(_BPC):
                    bfold_phase(bi)
                if split:
                    for bi in range(_BPC):
                        for h in range(2):
                            mm_phase(bi, h)
                    for bi in range(_BPC):
                        for h in range(2):
                            sc_phase(bi, h)
                            fin_phase(bi, h)
                else:
                    for bi in range(_BPC):
                        mm_phase(bi)
                    for bi in range(_BPC):
                        sc_phase(bi)
                        fin_phase(bi)

            if trips > 0:
                loop_kwargs = {}
                if hint:
                    loop_kwargs["hint_engines"] = (mybir.EngineType.PE,)
                if sr:
                    loop_kwargs["staggered_reset"] = True
                with tc.For_i(0, trips, 1, **loop_kwargs) as _i:
                    for _ in range(unroll):
                        body_once()
            for _ in range(tail):
                body_once()

            # --- writeback (fp16)
            for bi in range(_BPC):
                for g in range(_PL):
                    nc.sync.dma_start(out[bi, g], xs[bi][:, g, _IC : _IC + _N])

    nc.finalize()
    return nc


def parse_cfg(s: str) -> dict:
    """Parse 'ew0:ew1:unroll[:h][:s][:p][:sc][:bkN]' into a CONFIG dict."""
    parts = s.split(":")
    cfg = {"ew": (parts[0], parts[1]), "unroll": int(parts[2]), "hint": False,
           "sr": False, "split": False, "bk": 8, "sc": False, "fd": False,
           "ct": False}
    for p in parts[3:]:
        if p == "h":
            cfg["hint"] = True
        elif p == "s":
            cfg["sr"] = True
        elif p == "p":
            cfg["split"] = True
        elif p == "sc":
            cfg["sc"] = True
        elif p == "fd":
            cfg["fd"] = True
        elif p == "ct":
            cfg["ct"] = True
        elif p.startswith("bk"):
            cfg["bk"] = int(p[2:])
        else:
            raise ValueError(f"unknown config flag {p!r}")
    return cfg


_NC_CACHE: dict = {}


def _get_nc(maxiter: int):
    key = (maxiter, tuple(CONFIG["ew"]), CONFIG["unroll"],
           CONFIG.get("hint", False), CONFIG.get("sr", False),
           CONFIG.get("split", False), CONFIG.get("bk", 8),
           CONFIG.get("sc", False), CONFIG.get("fd", False),
           CONFIG.get("ct", False))
    if key not in _NC_CACHE:
        _NC_CACHE[key] = _build_nc(
            maxiter,
            ew_modes=tuple(CONFIG["ew"]),
            unroll=CONFIG["unroll"],
            hint=CONFIG.get("hint", False),
            sr=CONFIG.get("sr", False),
            split=CONFIG.get("split", False),
            bk=CONFIG.get("bk", 8),
            sc=CONFIG.get("sc", False),
            fd=CONFIG.get("fd", False),
            ct=CONFIG.get("ct", False),
        )
    return _NC_CACHE[key]


def _stencil_mats():
    # all stationaries pre-scaled by 0.25 (exact in fp16) so the PSUM
    # accumulator holds 0.25*(b + xN + xS) directly
    s = 0.25
    tm = np.zeros((_P, _P), np.float32)
    idx = np.arange(_P - 1)
    tm[idx, idx + 1] = s  # contribution of x[k] to out[k+1] (south nbr of k)
    tm[idx + 1, idx] = s  # north
    cn = np.zeros((_P, _P), np.float32)
    cn[_P - 1, 0] = s  # plane g-1 row 127 -> plane g row 0
    cs = np.zeros((_P, _P), np.float32)
    cs[0, _P - 1] = s  # plane g+1 row 0 -> plane g row 127
    im = s * np.eye(_P, dtype=np.float32)
    return tm, cn, cs, im


def _verify_stencil(M_rows, M_cols, M_vals, invD):
    """Check the COO matrix is exactly the uniform -1 4-neighbor stencil
    (no wraps) and invD == 0.25 everywhere."""
    r = np.asarray(M_rows).astype(np.int64)
    c = np.asarray(M_cols).astype(np.int64)
    v = np.asarray(M_vals)
    if not np.all(np.asarray(invD) == np.float32(0.25)):
        return False
    off = c - r
    bands = {}
    for o in (1, -1, _N, -_N):
        m = off == o
        bands[o] = m
    covered = bands[1] | bands[-1] | bands[_N] | bands[-_N]
    if not covered.all():
        return False
    # no row-wrap for the +-1 bands
    if np.any((r[bands[1]] % _N) == _N - 1) or np.any((r[bands[-1]] % _N) == 0):
        return False
    # each band must hit each eligible cell exactly once with value -1
    if not np.all(v == np.float32(-1.0)):
        return False
    n2 = _N * _N
    for o, m in bands.items():
        cnt = np.zeros(n2, np.int64)
        np.add.at(cnt, r[m], 1)
        rows2 = np.arange(n2)
        if o == 1:
            want = (rows2 % _N) != _N - 1
        elif o == -1:
            want = (rows2 % _N) != 0
        elif o == _N:
            want = rows2 < n2 - _N
        else:
            want = rows2 >= _N
        if not np.array_equal(cnt, want.astype(np.int64)):
            return False
    return True


def _fallback(u, b, M_rows, M_cols, M_vals, invD, maxiter):
    """Host scipy path — only taken if inputs are not the expected stencil."""
    from scipy.sparse import coo_matrix

    Bn = u.shape[0]
    n2 = _N * _N
    M = coo_matrix(
        (np.asarray(M_vals), (np.asarray(M_rows), np.asarray(M_cols))),
        shape=(n2, n2),
    ).tocsr()
    x = np.asarray(u).reshape(Bn, -1).astype(np.float32)
    bb = np.asarray(b).astype(np.float32)
    iD = np.asarray(invD).astype(np.float32)
    for _ in range(int(maxiter)):
        x = ((bb - (M @ x.T).T) * iD[None, :]).astype(np.float32)
    return x.reshape(u.shape)


TRACE = False
LAST = None  # BassKernelResults of the most recent run


def kernel(u, b, M_rows, M_cols, M_vals, invD, maxiter):
    global LAST
    from concourse.bass_utils import run_bass_kernel_spmd

    u = np.asarray(u)
    b = np.asarray(b)
    mi = int(maxiter)

    if not _verify_stencil(M_rows, M_cols, M_vals, invD):
        return _fallback(u, b, M_rows, M_cols, M_vals, invD, maxiter)

    nc = _get_nc(mi)
    tm, cn, cs, im = _stencil_mats()

    Bn = u.shape[0]
    assert Bn == _NCORES * _BPC
    u4 = np.ascontiguousarray(u.reshape(Bn, _PL, _P, _N).astype(np.float16))
    b4 = np.ascontiguousarray(b.reshape(Bn, _PL, _P, _N).astype(np.float16))

    in_maps = []
    for k in range(_NCORES):
        in_maps.append(
            {
                "u": u4[_BPC * k : _BPC * (k + 1)],
                "b": b4[_BPC * k : _BPC * (k + 1)],
                "tm": tm,
                "cn": cn,
                "cs": cs,
                "im": im,
            }
        )

    res = run_bass_kernel_spmd(nc, in_maps, list(range(_NCORES)), trace=TRACE)
    LAST = res
    outs = [res.results[k]["out"] for k in range(_NCORES)]
    full = np.concatenate(outs, axis=0).reshape(u.shape).astype(np.float32)
    return full


# revision 33
# speedup vs baseline: 1.2206x; 1.0114x over previous
"""Trainium2 Bass kernel for batched Jacobi iteration (5-point Laplacian).

Reference computation:
    x <- invD * (b - M x)   repeated `maxiter` times,
where M is the off-diagonal part of the 5-point Laplacian on a 512x512
grid, given in COO form.  For the actual inputs M is exactly the
4-neighbor stencil with value -1 and invD == 0.25, so the update is

    x_new[r, c] = 0.25 * (b[r, c] + x[r-1,c] + x[r+1,c] + x[r,c-1] + x[r,c+1])

(missing neighbors at grid edges contribute 0).

Strategy (8 NeuronCores, data parallel over batch B=16 -> 2 per core):
  - whole working set lives in SBUF for all iterations; state is fp16
  - grid stored as 4 "row planes" of (128 partitions=rows, 518 cols:
    2 zero pad cols each side so the interior starts 4B-aligned)
  - N/S coupling (+ optionally the b term) accumulates in PSUM via
    TensorE matmuls (tridiagonal / corner / identity stationaries,
    pre-scaled by 0.25)
  - E/W neighbor sum via shifted-AP adds split across DVE (plane 0,
    fp16 2x mode) and GpSimd (planes 1-3), double-buffered (bufs=2) so
    iteration k+1's ew overlaps iteration k's combine
  - b term: `bk` planes injected via TensorE identity matmuls, the rest
    folded into a GpSimd scalar_tensor_tensor with precomputed 0.25*b
  - optional `sc` mode: ScalarE copies PSUM->SBUF fp16 so the DVE
    combine runs in 2x packed mode
  - iterations run in a HARDWARE loop (tc.For_i) with a small unroll, so
    the program size is O(1) in maxiter
"""

import sys

sys.path.insert(0, "/opt/trn_rl_repo")

import numpy as np

_N = 512  # grid side
_PL = 4  # row planes per grid
_P = 128  # partitions
_IC = 2  # interior start col (2 zero pad cols each side, 4B aligned)
_W = _N + 2 * _IC  # padded row width
_NCORES = 8
_BPC = 2  # batches per core

# ew: per-batch engine for the E/W shifted add: "dve" | "gp" | "tensor"
# unroll: iterations per hardware-loop trip
# bk: number of grid planes (of 8 total = 2 batches x 4) whose b term is
#     injected on TensorE; the rest fold into a GpSimd stt with bq=0.25*b
# sc: ScalarE copies PSUM->fp16 SBUF; DVE combine runs 2x in SBUF
CONFIG = {"ew": ("mix", "mix"), "unroll": 10, "hint": False, "sr": False,
          "split": True, "bk": 8, "sc": False, "fd": False, "ct": False,
          "mk": 1}


def _build_nc(maxiter: int, ew_modes=("gp", "gp"), unroll=10, hint=False, sr=False,
              split=False, bk=8, sc=False, fd=False, ct=False, mk=2):
    import concourse.bacc as bacc
    import concourse.mybir as mybir
    from concourse.tile import TileContext

    f32 = mybir.dt.float32
    f16 = mybir.dt.float16
    nc = bacc.Bacc("TRN2", target_bir_lowering=False, debug=False, num_devices=_NCORES)

    u_in = nc.declare_dram_parameter("u", [_BPC, _PL, _P, _N], f16, isOutput=False)
    b_in = nc.declare_dram_parameter("b", [_BPC, _PL, _P, _N], f16, isOutput=False)
    tm_in = nc.declare_dram_parameter("tm", [_P, _P], f32, isOutput=False)
    cn_in = nc.declare_dram_parameter("cn", [_P, _P], f32, isOutput=False)
    cs_in = nc.declare_dram_parameter("cs", [_P, _P], f32, isOutput=False)
    im_in = nc.declare_dram_parameter("im", [_P, _P], f32, isOutput=False)
    out = nc.declare_dram_parameter("out", [_BPC, _PL, _P, _N], f16, isOutput=True)

    trips = maxiter // unroll
    tail = maxiter % unroll

    # per-batch count of b-planes on TensorE (batch 0 filled first)
    bk0 = min(_PL, bk)
    bk1 = min(_PL, bk - bk0)
    bks = (bk0, bk1)

    with TileContext(nc) as tc:
        with (
            tc.tile_pool(name="const", bufs=1) as const,
            tc.tile_pool(name="state", bufs=1) as state,
            tc.tile_pool(name="work", bufs=1) as work,
            tc.tile_pool(name="ewp", bufs=2) as ewp,
            tc.tile_pool(name="psum", bufs=1, space="PSUM") as psum,
        ):
            # --- stationaries: load f32, convert to fp16 (entries 0/0.25, exact)
            tmf = const.tile([_P, _P], f32, tag="tmf")
            cnf = const.tile([_P, _P], f32, tag="cnf")
            csf = const.tile([_P, _P], f32, tag="csf")
            imf = const.tile([_P, _P], f32, tag="imf")
            nc.sync.dma_start(tmf[:], tm_in[:])
            nc.sync.dma_start(cnf[:], cn_in[:])
            nc.sync.dma_start(csf[:], cs_in[:])
            nc.sync.dma_start(imf[:], im_in[:])
            tm = const.tile([_P, _P], f16, tag="tm")
            cn = const.tile([_P, _P], f16, tag="cn")
            cs = const.tile([_P, _P], f16, tag="cs")
            im = const.tile([_P, _P], f16, tag="im")
            for dst, src in ((tm, tmf), (cn, cnf), (cs, csf), (im, imf)):
                nc.vector.tensor_copy(dst[:], src[:])

            # --- per-batch state
            xs, bs, ews, pcs, ps = [], [], [], [], []
            for bi in range(_BPC):
                xh = state.tile([_P, _PL, _W], f16, tag=f"x{bi}")
                nc.gpsimd.memset(xh[:], 0.0)
                for g in range(_PL):
                    nc.sync.dma_start(xh[:, g, _IC : _IC + _N], u_in[bi, g])
                xs.append(xh)

                bh = state.tile([_P, _PL, _N], f16, tag=f"b{bi}")
                for g in range(_PL):
                    nc.sync.dma_start(bh[:, g, :], b_in[bi, g])
                bs.append(bh)

                ews.append(None)  # allocated per-iteration from ewp
                if sc:
                    pc = state.tile([_P, _PL, _N], f16, tag=f"pc{bi}")
                    pcs.append(pc)
                if split:
                    pa = psum.tile([_P, 2, _N], f32, tag=f"p{bi}a")
                    pb = psum.tile([_P, 2, _N], f32, tag=f"p{bi}b")
                    p = [pa, pb]
                else:
                    p = psum.tile([_P, _PL, _N], f32, tag=f"p{bi}")
                ps.append(p)

            def _psum_dst(bi, g):
                if split:
                    return ps[bi][g // 2][:, g % 2, :]
                return ps[bi][:, g, :]

            def _plane_mms(bi, g):
                """list of (lhsT_ap, rhs_ap, out_partition_slice) per plane"""
                xh = xs[bi]
                mms = []
                if g < bks[bi]:
                    mms.append((im[:], bs[bi][:, g, :], None))
                if ew_modes[bi] == "tensor":
                    mms.append((im[:], xh[:, g, _IC - 1 : _IC - 1 + _N], None))
                    mms.append((im[:], xh[:, g, _IC + 1 : _IC + 1 + _N], None))
                mms.append((tm[:], xh[:, g, _IC : _IC + _N], None))
                if g > 0:
                    if ct:
                        # corner hits out partition 0 only: M=32 col-tile 0
                        mms.append((cn[:, 0:32], xh[:, g - 1, _IC : _IC + _N],
                                    (0, 32)))
                    else:
                        mms.append((cn[:], xh[:, g - 1, _IC : _IC + _N], None))
                if g < _PL - 1:
                    if ct:
                        # corner hits out partition 127: M=32 col-tile 3
                        mms.append((cs[:, 96:128], xh[:, g + 1, _IC : _IC + _N],
                                    (96, 128)))
                    else:
                        mms.append((cs[:], xh[:, g + 1, _IC : _IC + _N], None))
                return mms

            def mm_phase(bi, half=None):
                gs = range(_PL) if half is None else range(2 * half, 2 * half + 2)
                if not ct:
                    for g in gs:
                        mms = _plane_mms(bi, g)
                        for i, (mat, rhs, osl) in enumerate(mms):
                            dst = _psum_dst(bi, g)
                            if osl is not None:
                                dst = dst[osl[0] : osl[1]]
                            nc.tensor.matmul(
                                dst, mat, rhs,
                                start=(i == 0), stop=(i == len(mms) - 1),
                            )
                    return
                # ct: b-MMs first (full-width start), then the M=32 corner
                # MMs clustered as grp0/grp3 pairs so they run concurrently,
                # then tm-MMs (full-width stop closes each bank's group)
                seq = []  # (mat, rhs, osl, bank)
                mains = {g: [] for g in gs}
                corners = []
                for g in gs:
                    for mat, rhs, osl in _plane_mms(bi, g):
                        if osl is not None:
                            corners.append((mat, rhs, osl, g))
                        else:
                            mains[g].append((mat, rhs, None, g))
                # pair-order: alternate grp0 (cn, osl 0:32) / grp3 (cs 96:128)
                corners.sort(key=lambda e: (e[3] + (0 if e[2][0] == 0 else 1)))
                for g in gs:
                    seq += mains[g][:-1]  # b (and tensor-ew) MMs
                seq += corners
                for g in gs:
                    seq.append(mains[g][-1])  # tm closes the bank
                first = {}
                last = {}
                for i, e in enumerate(seq):
                    first.setdefault(e[3], i)
                    last[e[3]] = i
                for i, (mat, rhs, osl, g) in enumerate(seq):
                    dst = _psum_dst(bi, g)
                    kw = {}
                    if osl is not None:
                        dst = dst[osl[0] : osl[1]]
                        kw["tile_position"] = (0, osl[0])
                    nc.tensor.matmul(
                        dst, mat, rhs,
                        start=(i == first[g]), stop=(i == last[g]), **kw,
                    )

            def ew_phase(bi):
                xh = xs[bi]
                mode = ew_modes[bi]
                if mode == "tensor":
                    return
                ew = ewp.tile([_P, _PL, _N], f16, tag=f"ew{bi}")
                ews[bi] = ew
                if mode == "mix":
                    # planes [0:mk] on DVE (fast 2x path), rest on GpSimd
                    nc.vector.tensor_add(
                        ew[:, 0:mk, :], xh[:, 0:mk, _IC - 1 : _IC - 1 + _N],
                        xh[:, 0:mk, _IC + 1 : _IC + 1 + _N]
                    )
                    nc.gpsimd.tensor_add(
                        ew[:, mk:_PL, :], xh[:, mk:_PL, _IC - 1 : _IC - 1 + _N],
                        xh[:, mk:_PL, _IC + 1 : _IC + 1 + _N]
                    )
                    return
                eng = nc.vector if mode == "dve" else nc.gpsimd
                eng.tensor_add(
                    ew[:], xh[:, :, _IC - 1 : _IC - 1 + _N],
                    xh[:, :, _IC + 1 : _IC + 1 + _N]
                )

            def bfold_phase(bi):
                """GpSimd: ew += b (in place) for planes whose b is folded;
                the fin stt's 0.25 scaling then covers b too."""
                k = bks[bi]
                if k >= _PL or ew_modes[bi] == "tensor":
                    return
                sl = slice(k, _PL)
                eng = nc.vector if fd else nc.gpsimd
                eng.tensor_add(
                    ews[bi][:, sl, :], ews[bi][:, sl, :], bs[bi][:, sl, :]
                )

            def sc_phase(bi, half=None):
                if not sc:
                    return
                if half is None:
                    nc.scalar.copy(pcs[bi][:], ps[bi][:])
                else:
                    sl = slice(2 * half, 2 * half + 2)
                    nc.scalar.copy(pcs[bi][:, sl, :], ps[bi][half][:])

            def fin_phase(bi, half=None):
                """combine into x: 0.25*ew + p (ew includes b on folded planes)"""
                xh = xs[bi]

                def _psrc(sl):
                    if sc:
                        return pcs[bi][:, sl, :]
                    if split:
                        h = sl.start // 2
                        return ps[bi][h][:, sl.start % 2 : sl.start % 2 + (sl.stop - sl.start), :]
                    return ps[bi][:, sl, :]

                lo, hi = (0, _PL) if half is None else (2 * half, 2 * half + 2)
                sl = slice(lo, hi)
                if ew_modes[bi] == "tensor":
                    nc.scalar.copy(xh[:, sl, _IC : _IC + _N], _psrc(sl))
                    return
                nc.vector.scalar_tensor_tensor(
                    xh[:, sl, _IC : _IC + _N], ews[bi][:, sl, :], 0.25,
                    _psrc(sl),
                    mybir.AluOpType.mult, mybir.AluOpType.add,
                )

            def body_once():
                for bi in range(_BPC):
                    ew_phase(bi)
                for bi in range(_BPC):
                    bfold_phase(bi)
                if split:
                    for bi in range(_BPC):
                        for h in range(2):
                            mm_phase(bi, h)
                    for bi in range(_BPC):
                        for h in range(2):
                            sc_phase(bi, h)
                            fin_phase(bi, h)
                else:
                    for bi in range(_BPC):
                        mm_phase(bi)
                    for bi in range(_BPC):
                        sc_phase(bi)
                        fin_phase(bi)

            if trips > 0:
                loop_kwargs = {}
                if hint:
                    loop_kwargs["hint_engines"] = (mybir.EngineType.PE,)
                if sr:
                    loop_kwargs["staggered_reset"] = True
                with tc.For_i(0, trips, 1, **loop_kwargs) as _i:
                    for _ in range(unroll):
                        body_once()
            for _ in range(tail):
                body_once()

            # --- writeback (fp16)
            for bi in range(_BPC):
                for g in range(_PL):
                    nc.sync.dma_start(out[bi, g], xs[bi][:, g, _IC : _IC + _N])

    nc.finalize()
    return nc


def parse_cfg(s: str) -> dict:
    """Parse 'ew0:ew1:unroll[:h][:s][:p][:sc][:bkN]' into a CONFIG dict."""
    parts = s.split(":")
    cfg = {"ew": (parts[0], parts[1]), "unroll": int(parts[2]), "hint": False,
           "sr": False, "split": False, "bk": 8, "sc": False, "fd": False,
           "ct": False, "mk": 2}
    for p in parts[3:]:
        if p == "h":
            cfg["hint"] = True
        elif p == "s":
            cfg["sr"] = True
        elif p == "p":
            cfg["split"] = True
        elif p == "sc":
            cfg["sc"] = True
        elif p == "fd":
            cfg["fd"] = True
        elif p == "ct":
            cfg["ct"] = True
        elif p.startswith("mk"):
            cfg["mk"] = int(p[2:])
        elif p.startswith("bk"):
            cfg["bk"] = int(p[2:])
        else:
            raise ValueError(f"unknown config flag {p!r}")
    return cfg


_NC_CACHE: dict = {}


def _get_nc(maxiter: int):
    key = (maxiter, tuple(CONFIG["ew"]), CONFIG["unroll"],
           CONFIG.get("hint", False), CONFIG.get("sr", False),
           CONFIG.get("split", False), CONFIG.get("bk", 8),
           CONFIG.get("sc", False), CONFIG.get("fd", False),
           CONFIG.get("ct", False), CONFIG.get("mk", 2))
    if key not in _NC_CACHE:
        _NC_CACHE[key] = _build_nc(
            maxiter,
            ew_modes=tuple(CONFIG["ew"]),
            unroll=CONFIG["unroll"],
            hint=CONFIG.get("hint", False),
            sr=CONFIG.get("sr", False),
            split=CONFIG.get("split", False),
            bk=CONFIG.get("bk", 8),
            sc=CONFIG.get("sc", False),
            fd=CONFIG.get("fd", False),
            ct=CONFIG.get("ct", False),
            mk=CONFIG.get("mk", 2),
        )
    return _NC_CACHE[key]


def _stencil_mats():
    # all stationaries pre-scaled by 0.25 (exact in fp16) so the PSUM
    # accumulator holds 0.25*(b + xN + xS) directly
    s = 0.25
    tm = np.zeros((_P, _P), np.float32)
    idx = np.arange(_P - 1)
    tm[idx, idx + 1] = s  # contribution of x[k] to out[k+1] (south nbr of k)
    tm[idx + 1, idx] = s  # north
    cn = np.zeros((_P, _P), np.float32)
    cn[_P - 1, 0] = s  # plane g-1 row 127 -> plane g row 0
    cs = np.zeros((_P, _P), np.float32)
    cs[0, _P - 1] = s  # plane g+1 row 0 -> plane g row 127
    im = s * np.eye(_P, dtype=np.float32)
    return tm, cn, cs, im


def _verify_stencil(M_rows, M_cols, M_vals, invD):
    """Check the COO matrix is exactly the uniform -1 4-neighbor stencil
    (no wraps) and invD == 0.25 everywhere."""
    r = np.asarray(M_rows).astype(np.int64)
    c = np.asarray(M_cols).astype(np.int64)
    v = np.asarray(M_vals)
    if not np.all(np.asarray(invD) == np.float32(0.25)):
        return False
    off = c - r
    bands = {}
    for o in (1, -1, _N, -_N):
        m = off == o
        bands[o] = m
    covered = bands[1] | bands[-1] | bands[_N] | bands[-_N]
    if not covered.all():
        return False
    # no row-wrap for the +-1 bands
    if np.any((r[bands[1]] % _N) == _N - 1) or np.any((r[bands[-1]] % _N) == 0):
        return False
    # each band must hit each eligible cell exactly once with value -1
    if not np.all(v == np.float32(-1.0)):
        return False
    n2 = _N * _N
    for o, m in bands.items():
        cnt = np.zeros(n2, np.int64)
        np.add.at(cnt, r[m], 1)
        rows2 = np.arange(n2)
        if o == 1:
            want = (rows2 % _N) != _N - 1
        elif o == -1:
            want = (rows2 % _N) != 0
        elif o == _N:
            want = rows2 < n2 - _N
        else:
            want = rows2 >= _N
        if not np.array_equal(cnt, want.astype(np.int64)):
            return False
    return True


def _fallback(u, b, M_rows, M_cols, M_vals, invD, maxiter):
    """Host scipy path — only taken if inputs are not the expected stencil."""
    from scipy.sparse import coo_matrix

    Bn = u.shape[0]
    n2 = _N * _N
    M = coo_matrix(
        (np.asarray(M_vals), (np.asarray(M_rows), np.asarray(M_cols))),
        shape=(n2, n2),
    ).tocsr()
    x = np.asarray(u).reshape(Bn, -1).astype(np.float32)
    bb = np.asarray(b).astype(np.float32)
    iD = np.asarray(invD).astype(np.float32)
    for _ in range(int(maxiter)):
        x = ((bb - (M @ x.T).T) * iD[None, :]).astype(np.float32)
    return x.reshape(u.shape)


TRACE = False
LAST = None  # BassKernelResults of the most recent run


def kernel(u, b, M_rows, M_cols, M_vals, invD, maxiter):
    global LAST
    from concourse.bass_utils import run_bass_kernel_spmd

    u = np.asarray(u)
    b = np.asarray(b)
    mi = int(maxiter)

    if not _verify_stencil(M_rows, M_cols, M_vals, invD):
        return _fallback(u, b, M_rows, M_cols, M_vals, invD, maxiter)

    nc = _get_nc(mi)
    tm, cn, cs, im = _stencil_mats()

    Bn = u.shape[0]
    assert Bn == _NCORES * _BPC
    u4 = np.ascontiguousarray(u.reshape(Bn, _PL, _P, _N).astype(np.float16))
    b4 = np.ascontiguousarray(b.reshape(Bn, _PL, _P, _N).astype(np.float16))

    in_maps = []
    for k in range(_NCORES):
        in_maps.append(
            {
                "u": u4[_BPC * k : _BPC * (k + 1)],
                "b": b4[_BPC * k : _BPC * (k + 1)],
                "tm": tm,
                "cn": cn,
                "cs": cs,
                "im": im,
            }
        )

    res = run_bass_kernel_spmd(nc, in_maps, list(range(_NCORES)), trace=TRACE)
    LAST = res
    outs = [res.results[k]["out"] for k in range(_NCORES)]
    full = np.concatenate(outs, axis=0).reshape(u.shape).astype(np.float32)
    return full


# revision 34
# speedup vs baseline: 1.2633x; 1.0350x over previous
"""Trainium2 Bass kernel for batched Jacobi iteration (5-point Laplacian).

Reference computation:
    x <- invD * (b - M x)   repeated `maxiter` times,
where M is the off-diagonal part of the 5-point Laplacian on a 512x512
grid, given in COO form.  For the actual inputs M is exactly the
4-neighbor stencil with value -1 and invD == 0.25, so the update is

    x_new[r, c] = 0.25 * (b[r, c] + x[r-1,c] + x[r+1,c] + x[r,c-1] + x[r,c+1])

(missing neighbors at grid edges contribute 0).

Strategy (8 NeuronCores, data parallel over batch B=16 -> 2 per core):
  - whole working set lives in SBUF for all iterations; state is fp16
  - grid stored as 4 "row planes" of (128 partitions=rows, 518 cols:
    2 zero pad cols each side so the interior starts 4B-aligned)
  - N/S coupling (+ optionally the b term) accumulates in PSUM via
    TensorE matmuls (tridiagonal / corner / identity stationaries,
    pre-scaled by 0.25)
  - E/W neighbor sum via shifted-AP adds split across DVE (plane 0,
    fp16 2x mode) and GpSimd (planes 1-3), double-buffered (bufs=2) so
    iteration k+1's ew overlaps iteration k's combine
  - b term: `bk` planes injected via TensorE identity matmuls, the rest
    folded into a GpSimd scalar_tensor_tensor with precomputed 0.25*b
  - optional `sc` mode: ScalarE copies PSUM->SBUF fp16 so the DVE
    combine runs in 2x packed mode
  - iterations run in a HARDWARE loop (tc.For_i) with a small unroll, so
    the program size is O(1) in maxiter
"""

import sys

sys.path.insert(0, "/opt/trn_rl_repo")

import numpy as np

_N = 512  # grid side
_PL = 4  # row planes per grid
_P = 128  # partitions
_IC = 2  # interior start col (2 zero pad cols each side, 4B aligned)
_W = _N + 2 * _IC  # padded row width
_NCORES = 8
_BPC = 2  # batches per core

# ew: per-batch engine for the E/W shifted add: "dve" | "gp" | "tensor"
# unroll: iterations per hardware-loop trip
# bk: number of grid planes (of 8 total = 2 batches x 4) whose b term is
#     injected on TensorE; the rest fold into a GpSimd stt with bq=0.25*b
# sc: ScalarE copies PSUM->fp16 SBUF; DVE combine runs 2x in SBUF
CONFIG = {"ew": ("mix", "mix"), "unroll": 10, "hint": False, "sr": False,
          "split": True, "bk": 8, "sc": False, "fd": False, "ct": False,
          "mk": (1, 1)}


def _build_nc(maxiter: int, ew_modes=("gp", "gp"), unroll=10, hint=False, sr=False,
              split=False, bk=8, sc=False, fd=False, ct=False, mk=(2, 2)):
    import concourse.bacc as bacc
    import concourse.mybir as mybir
    from concourse.tile import TileContext

    f32 = mybir.dt.float32
    f16 = mybir.dt.float16
    nc = bacc.Bacc("TRN2", target_bir_lowering=False, debug=False, num_devices=_NCORES)

    u_in = nc.declare_dram_parameter("u", [_BPC, _PL, _P, _N], f16, isOutput=False)
    b_in = nc.declare_dram_parameter("b", [_BPC, _PL, _P, _N], f16, isOutput=False)
    tm_in = nc.declare_dram_parameter("tm", [_P, _P], f32, isOutput=False)
    cn_in = nc.declare_dram_parameter("cn", [_P, _P], f32, isOutput=False)
    cs_in = nc.declare_dram_parameter("cs", [_P, _P], f32, isOutput=False)
    im_in = nc.declare_dram_parameter("im", [_P, _P], f32, isOutput=False)
    out = nc.declare_dram_parameter("out", [_BPC, _PL, _P, _N], f16, isOutput=True)

    trips = maxiter // unroll
    tail = maxiter % unroll

    # per-batch count of b-planes on TensorE (batch 0 filled first)
    bk0 = min(_PL, bk)
    bk1 = min(_PL, bk - bk0)
    bks = (bk0, bk1)

    with TileContext(nc) as tc:
        with (
            tc.tile_pool(name="const", bufs=1) as const,
            tc.tile_pool(name="state", bufs=1) as state,
            tc.tile_pool(name="work", bufs=1) as work,
            tc.tile_pool(name="ewp", bufs=2) as ewp,
            tc.tile_pool(name="psum", bufs=1, space="PSUM") as psum,
        ):
            # --- stationaries: load f32, convert to fp16 (entries 0/0.25, exact)
            tmf = const.tile([_P, _P], f32, tag="tmf")
            cnf = const.tile([_P, _P], f32, tag="cnf")
            csf = const.tile([_P, _P], f32, tag="csf")
            imf = const.tile([_P, _P], f32, tag="imf")
            nc.sync.dma_start(tmf[:], tm_in[:])
            nc.sync.dma_start(cnf[:], cn_in[:])
            nc.sync.dma_start(csf[:], cs_in[:])
            nc.sync.dma_start(imf[:], im_in[:])
            tm = const.tile([_P, _P], f16, tag="tm")
            cn = const.tile([_P, _P], f16, tag="cn")
            cs = const.tile([_P, _P], f16, tag="cs")
            im = const.tile([_P, _P], f16, tag="im")
            for dst, src in ((tm, tmf), (cn, cnf), (cs, csf), (im, imf)):
                nc.vector.tensor_copy(dst[:], src[:])

            # --- per-batch state
            xs, bs, ews, pcs, ps = [], [], [], [], []
            for bi in range(_BPC):
                xh = state.tile([_P, _PL, _W], f16, tag=f"x{bi}")
                nc.gpsimd.memset(xh[:], 0.0)
                for g in range(_PL):
                    nc.sync.dma_start(xh[:, g, _IC : _IC + _N], u_in[bi, g])
                xs.append(xh)

                bh = state.tile([_P, _PL, _N], f16, tag=f"b{bi}")
                for g in range(_PL):
                    nc.sync.dma_start(bh[:, g, :], b_in[bi, g])
                bs.append(bh)

                ews.append(None)  # allocated per-iteration from ewp
                if sc:
                    pc = state.tile([_P, _PL, _N], f16, tag=f"pc{bi}")
                    pcs.append(pc)
                if split:
                    pa = psum.tile([_P, 2, _N], f32, tag=f"p{bi}a")
                    pb = psum.tile([_P, 2, _N], f32, tag=f"p{bi}b")
                    p = [pa, pb]
                else:
                    p = psum.tile([_P, _PL, _N], f32, tag=f"p{bi}")
                ps.append(p)

            def _psum_dst(bi, g):
                if split:
                    return ps[bi][g // 2][:, g % 2, :]
                return ps[bi][:, g, :]

            def _plane_mms(bi, g):
                """list of (lhsT_ap, rhs_ap, out_partition_slice) per plane"""
                xh = xs[bi]
                mms = []
                if g < bks[bi]:
                    mms.append((im[:], bs[bi][:, g, :], None))
                if ew_modes[bi] == "tensor":
                    mms.append((im[:], xh[:, g, _IC - 1 : _IC - 1 + _N], None))
                    mms.append((im[:], xh[:, g, _IC + 1 : _IC + 1 + _N], None))
                mms.append((tm[:], xh[:, g, _IC : _IC + _N], None))
                if g > 0:
                    if ct:
                        # corner hits out partition 0 only: M=32 col-tile 0
                        mms.append((cn[:, 0:32], xh[:, g - 1, _IC : _IC + _N],
                                    (0, 32)))
                    else:
                        mms.append((cn[:], xh[:, g - 1, _IC : _IC + _N], None))
                if g < _PL - 1:
                    if ct:
                        # corner hits out partition 127: M=32 col-tile 3
                        mms.append((cs[:, 96:128], xh[:, g + 1, _IC : _IC + _N],
                                    (96, 128)))
                    else:
                        mms.append((cs[:], xh[:, g + 1, _IC : _IC + _N], None))
                return mms

            def mm_phase(bi, half=None):
                gs = range(_PL) if half is None else range(2 * half, 2 * half + 2)
                if not ct:
                    for g in gs:
                        mms = _plane_mms(bi, g)
                        for i, (mat, rhs, osl) in enumerate(mms):
                            dst = _psum_dst(bi, g)
                            if osl is not None:
                                dst = dst[osl[0] : osl[1]]
                            nc.tensor.matmul(
                                dst, mat, rhs,
                                start=(i == 0), stop=(i == len(mms) - 1),
                            )
                    return
                # ct: b-MMs first (full-width start), then the M=32 corner
                # MMs clustered as grp0/grp3 pairs so they run concurrently,
                # then tm-MMs (full-width stop closes each bank's group)
                seq = []  # (mat, rhs, osl, bank)
                mains = {g: [] for g in gs}
                corners = []
                for g in gs:
                    for mat, rhs, osl in _plane_mms(bi, g):
                        if osl is not None:
                            corners.append((mat, rhs, osl, g))
                        else:
                            mains[g].append((mat, rhs, None, g))
                # pair-order: alternate grp0 (cn, osl 0:32) / grp3 (cs 96:128)
                corners.sort(key=lambda e: (e[3] + (0 if e[2][0] == 0 else 1)))
                for g in gs:
                    seq += mains[g][:-1]  # b (and tensor-ew) MMs
                seq += corners
                for g in gs:
                    seq.append(mains[g][-1])  # tm closes the bank
                first = {}
                last = {}
                for i, e in enumerate(seq):
                    first.setdefault(e[3], i)
                    last[e[3]] = i
                for i, (mat, rhs, osl, g) in enumerate(seq):
                    dst = _psum_dst(bi, g)
                    kw = {}
                    if osl is not None:
                        dst = dst[osl[0] : osl[1]]
                        kw["tile_position"] = (0, osl[0])
                    nc.tensor.matmul(
                        dst, mat, rhs,
                        start=(i == first[g]), stop=(i == last[g]), **kw,
                    )

            def ew_phase(bi):
                xh = xs[bi]
                mode = ew_modes[bi]
                if mode == "tensor":
                    return
                ew = ewp.tile([_P, _PL, _N], f16, tag=f"ew{bi}")
                ews[bi] = ew
                if mode == "mix":
                    # planes [0:k] on DVE (fast 2x path), rest on GpSimd
                    k = mk[bi]
                    if k > 0:
                        nc.vector.tensor_add(
                            ew[:, 0:k, :], xh[:, 0:k, _IC - 1 : _IC - 1 + _N],
                            xh[:, 0:k, _IC + 1 : _IC + 1 + _N]
                        )
                    if k < _PL:
                        nc.gpsimd.tensor_add(
                            ew[:, k:_PL, :], xh[:, k:_PL, _IC - 1 : _IC - 1 + _N],
                            xh[:, k:_PL, _IC + 1 : _IC + 1 + _N]
                        )
                    return
                eng = nc.vector if mode == "dve" else nc.gpsimd
                eng.tensor_add(
                    ew[:], xh[:, :, _IC - 1 : _IC - 1 + _N],
                    xh[:, :, _IC + 1 : _IC + 1 + _N]
                )

            def bfold_phase(bi):
                """GpSimd: ew += b (in place) for planes whose b is folded;
                the fin stt's 0.25 scaling then covers b too."""
                k = bks[bi]
                if k >= _PL or ew_modes[bi] == "tensor":
                    return
                sl = slice(k, _PL)
                eng = nc.vector if fd else nc.gpsimd
                eng.tensor_add(
                    ews[bi][:, sl, :], ews[bi][:, sl, :], bs[bi][:, sl, :]
                )

            def sc_phase(bi, half=None):
                if not sc:
                    return
                if half is None:
                    nc.scalar.copy(pcs[bi][:], ps[bi][:])
                else:
                    sl = slice(2 * half, 2 * half + 2)
                    nc.scalar.copy(pcs[bi][:, sl, :], ps[bi][half][:])

            def fin_phase(bi, half=None):
                """combine into x: 0.25*ew + p (ew includes b on folded planes)"""
                xh = xs[bi]

                def _psrc(sl):
                    if sc:
                        return pcs[bi][:, sl, :]
                    if split:
                        h = sl.start // 2
                        return ps[bi][h][:, sl.start % 2 : sl.start % 2 + (sl.stop - sl.start), :]
                    return ps[bi][:, sl, :]

                lo, hi = (0, _PL) if half is None else (2 * half, 2 * half + 2)
                sl = slice(lo, hi)
                if ew_modes[bi] == "tensor":
                    nc.scalar.copy(xh[:, sl, _IC : _IC + _N], _psrc(sl))
                    return
                nc.vector.scalar_tensor_tensor(
                    xh[:, sl, _IC : _IC + _N], ews[bi][:, sl, :], 0.25,
                    _psrc(sl),
                    mybir.AluOpType.mult, mybir.AluOpType.add,
                )

            def body_once():
                for bi in range(_BPC):
                    ew_phase(bi)
                for bi in range(_BPC):
                    bfold_phase(bi)
                if split:
                    for bi in range(_BPC):
                        for h in range(2):
                            mm_phase(bi, h)
                    for bi in range(_BPC):
                        for h in range(2):
                            sc_phase(bi, h)
                            fin_phase(bi, h)
                else:
                    for bi in range(_BPC):
                        mm_phase(bi)
                    for bi in range(_BPC):
                        sc_phase(bi)
                        fin_phase(bi)

            if trips > 0:
                loop_kwargs = {}
                if hint:
                    loop_kwargs["hint_engines"] = (mybir.EngineType.PE,)
                if sr:
                    loop_kwargs["staggered_reset"] = True
                with tc.For_i(0, trips, 1, **loop_kwargs) as _i:
                    for _ in range(unroll):
                        body_once()
            for _ in range(tail):
                body_once()

            # --- writeback (fp16)
            for bi in range(_BPC):
                for g in range(_PL):
                    nc.sync.dma_start(out[bi, g], xs[bi][:, g, _IC : _IC + _N])

    nc.finalize()
    return nc


def parse_cfg(s: str) -> dict:
    """Parse 'ew0:ew1:unroll[:h][:s][:p][:sc][:bkN]' into a CONFIG dict."""
    parts = s.split(":")
    cfg = {"ew": (parts[0], parts[1]), "unroll": int(parts[2]), "hint": False,
           "sr": False, "split": False, "bk": 8, "sc": False, "fd": False,
           "ct": False, "mk": (2, 2)}
    for p in parts[3:]:
        if p == "h":
            cfg["hint"] = True
        elif p == "s":
            cfg["sr"] = True
        elif p == "p":
            cfg["split"] = True
        elif p == "sc":
            cfg["sc"] = True
        elif p == "fd":
            cfg["fd"] = True
        elif p == "ct":
            cfg["ct"] = True
        elif p.startswith("mk"):
            ds = p[2:]
            cfg["mk"] = (int(ds[0]), int(ds[1])) if len(ds) == 2 else (int(ds), int(ds))
        elif p.startswith("bk"):
            cfg["bk"] = int(p[2:])
        else:
            raise ValueError(f"unknown config flag {p!r}")
    return cfg


_NC_CACHE: dict = {}


def _get_nc(maxiter: int):
    key = (maxiter, tuple(CONFIG["ew"]), CONFIG["unroll"],
           CONFIG.get("hint", False), CONFIG.get("sr", False),
           CONFIG.get("split", False), CONFIG.get("bk", 8),
           CONFIG.get("sc", False), CONFIG.get("fd", False),
           CONFIG.get("ct", False), tuple(CONFIG.get("mk", (2, 2))))
    if key not in _NC_CACHE:
        _NC_CACHE[key] = _build_nc(
            maxiter,
            ew_modes=tuple(CONFIG["ew"]),
            unroll=CONFIG["unroll"],
            hint=CONFIG.get("hint", False),
            sr=CONFIG.get("sr", False),
            split=CONFIG.get("split", False),
            bk=CONFIG.get("bk", 8),
            sc=CONFIG.get("sc", False),
            fd=CONFIG.get("fd", False),
            ct=CONFIG.get("ct", False),
            mk=tuple(CONFIG.get("mk", (2, 2))),
        )
    return _NC_CACHE[key]


def _stencil_mats():
    # all stationaries pre-scaled by 0.25 (exact in fp16) so the PSUM
    # accumulator holds 0.25*(b + xN + xS) directly
    s = 0.25
    tm = np.zeros((_P, _P), np.float32)
    idx = np.arange(_P - 1)
    tm[idx, idx + 1] = s  # contribution of x[k] to out[k+1] (south nbr of k)
    tm[idx + 1, idx] = s  # north
    cn = np.zeros((_P, _P), np.float32)
    cn[_P - 1, 0] = s  # plane g-1 row 127 -> plane g row 0
    cs = np.zeros((_P, _P), np.float32)
    cs[0, _P - 1] = s  # plane g+1 row 0 -> plane g row 127
    im = s * np.eye(_P, dtype=np.float32)
    return tm, cn, cs, im


def _verify_stencil(M_rows, M_cols, M_vals, invD):
    """Check the COO matrix is exactly the uniform -1 4-neighbor stencil
    (no wraps) and invD == 0.25 everywhere."""
    r = np.asarray(M_rows).astype(np.int64)
    c = np.asarray(M_cols).astype(np.int64)
    v = np.asarray(M_vals)
    if not np.all(np.asarray(invD) == np.float32(0.25)):
        return False
    off = c - r
    bands = {}
    for o in (1, -1, _N, -_N):
        m = off == o
        bands[o] = m
    covered = bands[1] | bands[-1] | bands[_N] | bands[-_N]
    if not covered.all():
        return False
    # no row-wrap for the +-1 bands
    if np.any((r[bands[1]] % _N) == _N - 1) or np.any((r[bands[-1]] % _N) == 0):
        return False
    # each band must hit each eligible cell exactly once with value -1
    if not np.all(v == np.float32(-1.0)):
        return False
    n2 = _N * _N
    for o, m in bands.items():
        cnt = np.zeros(n2, np.int64)
        np.add.at(cnt, r[m], 1)
        rows2 = np.arange(n2)
        if o == 1:
            want = (rows2 % _N) != _N - 1
        elif o == -1:
            want = (rows2 % _N) != 0
        elif o == _N:
            want = rows2 < n2 - _N
        else:
            want = rows2 >= _N
        if not np.array_equal(cnt, want.astype(np.int64)):
            return False
    return True


def _fallback(u, b, M_rows, M_cols, M_vals, invD, maxiter):
    """Host scipy path — only taken if inputs are not the expected stencil."""
    from scipy.sparse import coo_matrix

    Bn = u.shape[0]
    n2 = _N * _N
    M = coo_matrix(
        (np.asarray(M_vals), (np.asarray(M_rows), np.asarray(M_cols))),
        shape=(n2, n2),
    ).tocsr()
    x = np.asarray(u).reshape(Bn, -1).astype(np.float32)
    bb = np.asarray(b).astype(np.float32)
    iD = np.asarray(invD).astype(np.float32)
    for _ in range(int(maxiter)):
        x = ((bb - (M @ x.T).T) * iD[None, :]).astype(np.float32)
    return x.reshape(u.shape)


TRACE = False
LAST = None  # BassKernelResults of the most recent run


def kernel(u, b, M_rows, M_cols, M_vals, invD, maxiter):
    global LAST
    from concourse.bass_utils import run_bass_kernel_spmd

    u = np.asarray(u)
    b = np.asarray(b)
    mi = int(maxiter)

    if not _verify_stencil(M_rows, M_cols, M_vals, invD):
        return _fallback(u, b, M_rows, M_cols, M_vals, invD, maxiter)

    nc = _get_nc(mi)
    tm, cn, cs, im = _stencil_mats()

    Bn = u.shape[0]
    assert Bn == _NCORES * _BPC
    u4 = np.ascontiguousarray(u.reshape(Bn, _PL, _P, _N).astype(np.float16))
    b4 = np.ascontiguousarray(b.reshape(Bn, _PL, _P, _N).astype(np.float16))

    in_maps = []
    for k in range(_NCORES):
        in_maps.append(
            {
                "u": u4[_BPC * k : _BPC * (k + 1)],
                "b": b4[_BPC * k : _BPC * (k + 1)],
                "tm": tm,
                "cn": cn,
                "cs": cs,
                "im": im,
            }
        )

    res = run_bass_kernel_spmd(nc, in_maps, list(range(_NCORES)), trace=TRACE)
    LAST = res
    outs = [res.results[k]["out"] for k in range(_NCORES)]
    full = np.concatenate(outs, axis=0).reshape(u.shape).astype(np.float32)
    return full
